# revision 9
# baseline (speedup 1.0000x reference)
"""Multi-head attention (SEQ=4096, EMBED=1024, 16 heads, Dh=64) on 8 TRN2
NeuronCores, head-parallel: 2 heads per core, Wo row-sharded so each core
emits a partial output [SEQ, EMBED]; the host sums the 8 partials (+bo).

Per-core pipeline (one TileContext):
  A) hidden^T e-chunks via PE transposes (fp32 has no DMA transpose);
     projections Q^T,K^T [128(=2 heads x 64 d), S] and V' [t, 2*(64+ones)]
     with the bias fused into the PSUM->SBUF copy. The 1/sqrt(Dh) scale is
     folded into Wq/bq on the host (exact: *0.125).
  B) per s-super(512): scores^T [t, s] as two row-tiled K=64 matmuls (the
     two heads concurrently fill the 128-row array, tile_position (0,0) /
     (64,0)); one Exp per t-chunk over the [128, 2*512] PSUM pair (logits
     are bounded ~|3|, so no max subtraction needed); attn'^T [65, s]
     accumulated in PSUM where the ones column of V' makes row 64 the
     softmax denominator.
  C) denominators -> tiny PE transposes -> DVE reciprocal (s-partitioned);
     Wo per head into separate PSUM banks; DVE scales each head's partial
     by its 1/D and sums; DMA out.
"""

import os
import sys

sys.path.insert(0, "/opt/trn_rl_repo")

import numpy as np

SEQ = 4096
EMBED = 1024
HEADS = 16
HD = 64
NCORES = 8
HPC = HEADS // NCORES  # 2 heads per core
EC = EMBED // 128  # 8 e-chunks
SUP = 512  # s-super size
NSUP = SEQ // SUP  # 8
TC = SEQ // 128  # 32 t-chunks
JS = SUP // 128  # 4 s-tiles per super

# dtype knobs: f32r = fp32 storage, single-pass reduced-precision matmul
MM_DT = os.environ.get("K_MM_DT", "f32r")  # f32r | f32
PV_DT = os.environ.get("K_PV_DT", "f32")  # f32 | bf16 (P^T/V'/x^T/Wo storage)

LAST = None  # BassKernelResults of the most recent run (read by test.py)
_CACHE = {}


def _mm(ap):
    """View an fp32 AP as float32r for single-pass matmuls, if enabled."""
    from concourse import mybir

    if MM_DT == "f32r" and ap.dtype == mybir.dt.float32:
        return ap.bitcast(mybir.dt.float32r)
    return ap


def _build():
    import concourse.bacc as bacc
    import concourse.tile as tile
    from concourse import mybir

    f32 = mybir.dt.float32

    nc = bacc.Bacc("TRN2", debug=False, enable_asserts=False, num_devices=NCORES)

    hid = nc.dram_tensor("hidden", [SEQ, EMBED], f32, kind="ExternalInput").ap()
    wqkv = nc.dram_tensor("w_qkv", [3, EC, 128, 128], f32, kind="ExternalInput").ap()
    bqk = nc.dram_tensor("b_qk", [2, 128], f32, kind="ExternalInput").ap()
    bv = nc.dram_tensor("b_v", [128], f32, kind="ExternalInput").ap()
    wo = nc.dram_tensor("w_o", [64, 2, EMBED], f32, kind="ExternalInput").ap()
    ident = nc.dram_tensor("ident", [128, 128], f32, kind="ExternalInput").ap()
    outp = nc.dram_tensor("out_p", [SEQ, EMBED], f32, kind="ExternalOutput").ap()

    with tile.TileContext(nc) as tc:
        _emit(tc, mybir, hid, wqkv, bqk, bv, wo, ident, outp)

    nc.compile()
    return nc


def _emit(tc, mybir, hid, wqkv, bqk, bv, wo, ident, outp):
    import concourse.bass as bass

    nc = tc.nc
    ts = bass.ts
    f32 = mybir.dt.float32
    f32r = mybir.dt.float32r
    mmdt = f32r if MM_DT == "f32r" else f32
    # attention-side storage: bf16 if requested, else the matmul fp32 flavor
    pv = mybir.dt.bfloat16 if PV_DT == "bf16" else mmdt
    Exp = mybir.ActivationFunctionType.Exp
    AOT = mybir.AluOpType

    # ---- persistent tiles -------------------------------------------------
    import contextlib

    _stack = contextlib.ExitStack()
    persist = _stack.enter_context(tc.tile_pool(name="persist", bufs=1))
    qT = persist.tile([128, SEQ], pv, tag="qT")  # [(h,d), s]
    kT = persist.tile([128, SEQ], pv, tag="kT")
    vP = persist.tile([128, TC, 2 * (HD + 1)], pv, tag="vP")  # V' per t-chunk
    xT = [persist.tile([HD + 1, SEQ], pv, tag=f"xT{h}", name=f"xT{h}") for h in range(HPC)]
    wq_sb = persist.tile([128, EC, 128], mmdt, tag="wq")
    wk_sb = persist.tile([128, EC, 128], mmdt, tag="wk")
    wv_sb = persist.tile([128, EC, 128], mmdt, tag="wv")
    wo_sb = persist.tile([64, 2, EMBED], pv, tag="wo")
    id_sb = persist.tile([128, 128], f32, tag="ident")
    bq_sb = persist.tile([128, 1], f32, tag="bq")
    bk_sb = persist.tile([128, 1], f32, tag="bk")
    bv_sb = persist.tile([128, 128], f32, tag="bv")

    w_stage = persist.tile([128, 3, EC, 128], f32, tag="w_stage")
    nc.sync.dma_start(out=w_stage, in_=wqkv.rearrange("w c e d -> e w c d"))
    for i, w_sb in enumerate((wq_sb, wk_sb, wv_sb)):
        if w_sb.dtype == f32:
            nc.sync.dma_start(out=w_sb, in_=wqkv[i].rearrange("c e d -> e c d"))
        else:
            nc.vector.tensor_copy(out=w_sb, in_=w_stage[:, i])
    if pv == f32:
        nc.sync.dma_start(out=wo_sb, in_=wo)
    else:
        wo_f32 = persist.tile([64, 2, EMBED], f32, tag="wo_f32")
        nc.sync.dma_start(out=wo_f32, in_=wo)
        nc.vector.tensor_copy(out=wo_sb, in_=wo_f32)
    nc.sync.dma_start(out=id_sb, in_=ident)
    nc.sync.dma_start(out=bq_sb, in_=bqk[0:1, :].rearrange("a p -> p a"))
    nc.sync.dma_start(out=bk_sb, in_=bqk[1:2, :].rearrange("a p -> p a"))
    bv_bcast = bass.AP(tensor=bv.tensor, offset=bv.offset, ap=[[0, 128], [1, 128]])
    nc.sync.dma_start(out=bv_sb, in_=bv_bcast)
    # ones columns of V' (free positions h*65+64); V overwrites cols 0..63
    ones_sb = persist.tile([128, 1], f32, tag="ones")
    nc.vector.memset(ones_sb, 1.0)
    vP_ones = vP.rearrange("p c (h e) -> p c h e", h=2)[:, :, :, HD : HD + 1]
    ones_b = bass.AP(
        tensor=ones_sb.tensor,
        offset=ones_sb.offset,
        ap=[ones_sb.ap[0], [0, TC], [0, 2], [0, 1]],
    )
    nc.vector.tensor_copy(out=vP_ones, in_=ones_b)

    # ---- phase A: hidden^T + projections ---------------------------------
    with (
        tc.tile_pool(name="hnat", bufs=4) as hnat_p,
        tc.tile_pool(name="hT", bufs=2) as hT_p,
        tc.tile_pool(name="ps_tp", bufs=2, space="PSUM") as tp_ps_p,
        tc.tile_pool(name="ps_proj", bufs=2, space="PSUM") as proj_ps_p,
    ):
        for sb in range(NSUP):  # s-blocks of 512
            hn = [hnat_p.tile([128, EMBED], f32, tag="hn", name=f"hn{sb}_{_j}") for _j in range(JS)]
            for j in range(JS):
                nc.sync.dma_start(out=hn[j], in_=hid[ts(JS * sb + j, 128), :])
            hT = hT_p.tile([128, EC, SUP], mmdt, tag="hT")  # [e, chunk, s]
            for c in range(EC):
                tp = tp_ps_p.tile([128, SUP], f32, tag="tp")
                for j in range(JS):
                    nc.tensor.transpose(
                        tp[:, ts(j, 128)], hn[j][:, ts(c, 128)], id_sb
                    )
                nc.vector.tensor_copy(out=hT[:, c, :], in_=tp)
            q_ps = proj_ps_p.tile([128, SUP], f32, tag="q_ps")
            k_ps = proj_ps_p.tile([128, SUP], f32, tag="k_ps")
            v_ps = proj_ps_p.tile([128, SUP], f32, tag="v_ps")
            for c in range(EC):
                nc.tensor.matmul(
                    q_ps, _mm(wq_sb[:, c, :]), _mm(hT[:, c, :]),
                    start=(c == 0), stop=(c == EC - 1),
                )
            for c in range(EC):
                nc.tensor.matmul(
                    k_ps, _mm(wk_sb[:, c, :]), _mm(hT[:, c, :]),
                    start=(c == 0), stop=(c == EC - 1),
                )
            for j in range(JS):  # V natural [t, d], 4 t-tiles per block
                for c in range(EC):
                    nc.tensor.matmul(
                        v_ps[:, ts(j, 128)],
                        _mm(hT[:, c, ts(j, 128)]),
                        _mm(wv_sb[:, c, :]),
                        start=(c == 0), stop=(c == EC - 1),
                        skip_group_check=True,
                    )
            # PSUM -> SBUF with fused bias (and optional bf16 downcast)
            nc.vector.tensor_scalar(
                out=qT[:, ts(sb, SUP)], in0=q_ps,
                scalar1=bq_sb, scalar2=None, op0=AOT.add,
            )
            nc.vector.tensor_scalar(
                out=kT[:, ts(sb, SUP)], in0=k_ps,
                scalar1=bk_sb, scalar2=None, op0=AOT.add,
            )
            for j in range(JS):
                t_idx = JS * sb + j
                dst = vP[:, t_idx, :].rearrange("p (h e) -> p h e", h=2)[:, :, 0:HD]
                nc.vector.tensor_add(
                    out=dst,
                    in0=v_ps[:, ts(j, 128)].rearrange("p (h d) -> p h d", h=2),
                    in1=bv_sb.rearrange("p (h d) -> p h d", h=2),
                )

    # ---- phases B+C: attention + output, per s-super ---------------------
    with (
        tc.tile_pool(name="pT", bufs=2) as pT_p,
        tc.tile_pool(name="dtmp", bufs=2) as dtmp_p,
        tc.tile_pool(name="rD", bufs=2) as rD_p,
        tc.tile_pool(name="anorm", bufs=3) as an_p,
        tc.tile_pool(name="stage", bufs=2) as st_p,
        tc.tile_pool(name="ps_sc", bufs=2, space="PSUM") as sc_ps_p,
        tc.tile_pool(name="ps_at", bufs=1, space="PSUM") as at_ps_p,
        tc.tile_pool(name="ps_wo", bufs=1, space="PSUM") as wo_ps_p,
    ):
        for sup in range(NSUP):
            ssl = ts(sup, SUP)
            at_ps = [
                at_ps_p.tile([HD + 1, SUP], f32, tag=f"at{h}", name=f"at{sup}_{h}") for h in range(HPC)
            ]
            for c in range(TC):
                sc_ps = sc_ps_p.tile([128, 2 * SUP], f32, tag="sc")
                for h in range(HPC):
                    nc.tensor.matmul(
                        sc_ps[:, ts(h, SUP)],
                        _mm(kT[ts(h, HD), ts(c, 128)]),
                        _mm(qT[ts(h, HD), ssl]),
                        start=True, stop=True,
                        tile_position=(h * HD, 0),
                    )
                pT = pT_p.tile([128, 2 * SUP], pv, tag="pT")
                nc.scalar.activation(out=pT, in_=sc_ps, func=Exp)
                for h in range(HPC):
                    nc.tensor.matmul(
                        at_ps[h],
                        _mm(vP[:, c, ts(h, HD + 1)]),
                        _mm(pT[:, ts(h, SUP)]),
                        start=(c == 0), stop=(c == TC - 1),
                    )
            # attn'^T -> SBUF; row 64 is the softmax denominator
            d_tmp = [
                dtmp_p.tile([1, SUP], f32, tag=f"d{h}", name=f"d{sup}_{h}")
                for h in range(HPC)
            ]
            for h in range(HPC):
                nc.vector.tensor_copy(out=xT[h][:, ssl], in_=at_ps[h])
                nc.vector.tensor_copy(
                    out=d_tmp[h], in_=at_ps[h][HD : HD + 1, :]
                )
            # denominators -> s-partitioned layout via tiny PE transposes
            dT_ps = sc_ps_p.tile([128, HPC * JS], f32, tag="sc")
            for h in range(HPC):
                for j in range(JS):
                    nc.tensor.transpose(
                        dT_ps[:, h * JS + j : h * JS + j + 1],
                        d_tmp[h][:, ts(j, 128)],
                        id_sb[0:1, 0:1],
                    )
            rD = rD_p.tile([128, HPC, JS], f32, tag="rD")
            nc.vector.reciprocal(
                out=rD.rearrange("p h j -> p (h j)"), in_=dT_ps
            )
            # Wo per head, then scale by 1/D_h and add heads
            for j in range(JS):
                st_i = JS * sup + j
                stage = st_p.tile([128, EMBED], f32, tag="stage")
                for eh in range(EMBED // SUP):
                    o_ps = [
                        wo_ps_p.tile([128, SUP], f32, tag=f"o{h}", name=f"o{st_i}_{eh}_{h}")
                        for h in range(HPC)
                    ]
                    for h in range(HPC):
                        nc.tensor.matmul(
                            o_ps[h],
                            _mm(xT[h][0:HD, ts(st_i, 128)]),
                            _mm(wo_sb[:, h, ts(eh, SUP)]),
                            start=True, stop=True,
                        )
                    t0 = an_p.tile([128, SUP], f32, tag="t0")
                    t1 = an_p.tile([128, SUP], f32, tag="t1")
                    nc.vector.tensor_scalar_mul(
                        out=t0, in0=o_ps[0], scalar1=rD[:, 0, j : j + 1]
                    )
                    nc.vector.tensor_scalar_mul(
                        out=t1, in0=o_ps[1], scalar1=rD[:, 1, j : j + 1]
                    )
                    nc.vector.tensor_add(
                        out=stage[:, ts(eh, SUP)], in0=t0, in1=t1
                    )
                nc.sync.dma_start(out=outp[ts(st_i, 128), :], in_=stage)

    _stack.close()


def _shards(inputs):
    """Host-side prep: per-core input dicts (head-parallel, Wo row-shard)."""
    hs = np.ascontiguousarray(np.asarray(inputs["hidden_state"], np.float32))
    Wq = np.asarray(inputs["Wq"], np.float32) * 0.125  # fold 1/sqrt(64); exact
    bq = np.asarray(inputs["bq"], np.float32) * 0.125
    Wk = np.asarray(inputs["Wk"], np.float32)
    bk = np.asarray(inputs["bk"], np.float32)
    Wv = np.asarray(inputs["Wv"], np.float32)
    bv = np.asarray(inputs["bv"], np.float32)
    Wo = np.asarray(inputs["Wo"], np.float32)
    ident = np.eye(128, dtype=np.float32)

    in_maps = []
    for c in range(NCORES):
        h0 = HPC * c
        # [H,E,Dh] head-pair -> [E, 2*Dh] -> [EC, 128, 128]
        def _w(W):
            w = np.transpose(W[h0 : h0 + HPC], (1, 0, 2)).reshape(EMBED, 128)
            return np.ascontiguousarray(w.reshape(EC, 128, 128))

        w_qkv = np.stack([_w(Wq), _w(Wk), _w(Wv)])
        b_qk = np.stack(
            [bq[h0 : h0 + HPC].reshape(128), bk[h0 : h0 + HPC].reshape(128)]
        )
        b_v = np.ascontiguousarray(bv[h0 : h0 + HPC].reshape(128))
        # Wo rows for this core's heads: [128, E] -> [64, 2, E] (head-major)
        w_o = np.ascontiguousarray(
            Wo[128 * c : 128 * (c + 1)].reshape(2, 64, EMBED).transpose(1, 0, 2)
        )
        in_maps.append(
            {
                "hidden": hs,
                "w_qkv": np.ascontiguousarray(w_qkv),
                "b_qk": np.ascontiguousarray(b_qk),
                "b_v": b_v,
                "w_o": w_o,
                "ident": ident,
            }
        )
    return in_maps


def kernel(**inputs):
    global LAST
    from concourse import bass_utils

    trace = bool(int(os.environ.get("K_TRACE", "0")))
    if trace:
        _install_ntff_shim()

    key = (MM_DT, PV_DT)
    if key not in _CACHE:
        _CACHE[key] = _build()
    nc = _CACHE[key]

    in_maps = _shards(inputs)
    res = bass_utils.run_bass_kernel_spmd(
        nc, in_maps, core_ids=list(range(NCORES)), trace=trace
    )
    LAST = res

    out = np.zeros((SEQ, EMBED), np.float64)
    for c in range(NCORES):
        out += res.results[c]["out_p"].astype(np.float64)
    out += np.asarray(inputs["bo"], np.float32).astype(np.float64)
    return out.astype(np.float32)


def _install_ntff_shim():
    """antenv.axon_hooks is absent from this image; recreate it so
    run_bass_kernel_spmd(trace=True) can reach the NTFF profiling hook."""
    import types

    if "antenv.axon_hooks" in sys.modules:
        return
    try:
        if "/root/.axon_site" not in sys.path:
            sys.path.insert(0, "/root/.axon_site")
        from trn_agent_boot.trn_boot import _ntff_profile_via_ctypes

        hook = _ntff_profile_via_ctypes("/opt/axon/libaxon_pjrt.so")
    except Exception:
        hook = None
    mod = types.ModuleType("antenv.axon_hooks")
    mod._hook = hook
    mod.get_axon_ntff_profile_hook = lambda: mod._hook
    mod.set_axon_ntff_profile_hook = lambda h: setattr(mod, "_hook", h)
    sys.modules["antenv.axon_hooks"] = mod


# revision 10
# speedup vs baseline: 1.2298x; 1.2298x over previous
"""Multi-head attention (SEQ=4096, EMBED=1024, 16 heads, Dh=64) on 8 TRN2
NeuronCores, head-parallel: 2 heads per core, Wo row-sharded so each core
emits a partial output [SEQ, EMBED]; the host sums the 8 partials (+bo).

Per-core pipeline (one TileContext):
  A) hidden^T e-chunks via PE transposes (fp32 has no DMA transpose);
     projections Q^T,K^T [128(=2 heads x 64 d), S] and V' [t, 2*(64+ones)]
     with the bias fused into the PSUM->SBUF copy. The 1/sqrt(Dh) scale is
     folded into Wq/bq on the host (exact: *0.125).
  B) per s-super(512): scores^T [t, s] as two row-tiled K=64 matmuls (the
     two heads concurrently fill the 128-row array, tile_position (0,0) /
     (64,0)); one Exp per t-chunk over the [128, 2*512] PSUM pair (logits
     are bounded ~|3|, so no max subtraction needed); attn'^T [65, s]
     accumulated in PSUM where the ones column of V' makes row 64 the
     softmax denominator.
  C) denominators -> tiny PE transposes -> DVE reciprocal (s-partitioned);
     Wo per head into separate PSUM banks; DVE scales each head's partial
     by its 1/D and sums; DMA out.
"""

import os
import sys

sys.path.insert(0, "/opt/trn_rl_repo")

import numpy as np

SEQ = 4096
EMBED = 1024
HEADS = 16
HD = 64
NCORES = 8
HPC = HEADS // NCORES  # 2 heads per core
EC = EMBED // 128  # 8 e-chunks
SUP = 512  # s-super size
NSUP = SEQ // SUP  # 8
TC = SEQ // 128  # 32 t-chunks
JS = SUP // 128  # 4 s-tiles per super

# dtype knobs: f32r = fp32 storage, single-pass reduced-precision matmul
MM_DT = os.environ.get("K_MM_DT", "f32r")  # f32r | f32
PV_DT = os.environ.get("K_PV_DT", "f32")  # f32 | bf16 (P^T/V'/x^T/Wo storage)

LAST = None  # BassKernelResults of the most recent run (read by test.py)
_CACHE = {}


def _mm(ap):
    """View an fp32 AP as float32r for single-pass matmuls, if enabled."""
    from concourse import mybir

    if MM_DT == "f32r" and ap.dtype == mybir.dt.float32:
        return ap.bitcast(mybir.dt.float32r)
    return ap


def _build():
    import concourse.bacc as bacc
    import concourse.tile as tile
    from concourse import mybir

    f32 = mybir.dt.float32

    nc = bacc.Bacc("TRN2", debug=False, enable_asserts=False, num_devices=NCORES)

    hid = nc.dram_tensor("hidden", [SEQ, EMBED], f32, kind="ExternalInput").ap()
    wqkv = nc.dram_tensor("w_qkv", [3, EC, 128, 128], f32, kind="ExternalInput").ap()
    bqk = nc.dram_tensor("b_qk", [2, 128], f32, kind="ExternalInput").ap()
    bv = nc.dram_tensor("b_v", [128], f32, kind="ExternalInput").ap()
    wo = nc.dram_tensor("w_o", [64, 2, EMBED], f32, kind="ExternalInput").ap()
    ident = nc.dram_tensor("ident", [128, 128], f32, kind="ExternalInput").ap()
    outp = nc.dram_tensor("out_p", [SEQ, EMBED], f32, kind="ExternalOutput").ap()

    with tile.TileContext(nc) as tc:
        _emit(tc, mybir, hid, wqkv, bqk, bv, wo, ident, outp)

    nc.compile()
    return nc


def _emit(tc, mybir, hid, wqkv, bqk, bv, wo, ident, outp):
    import concourse.bass as bass

    nc = tc.nc
    ts = bass.ts
    f32 = mybir.dt.float32
    f32r = mybir.dt.float32r
    mmdt = f32r if MM_DT == "f32r" else f32
    # attention-side storage: bf16 if requested, else the matmul fp32 flavor
    pv = mybir.dt.bfloat16 if PV_DT == "bf16" else mmdt
    projdt = mybir.dt.bfloat16 if PV_DT == "bf16" else mmdt
    Exp = mybir.ActivationFunctionType.Exp
    AOT = mybir.AluOpType

    # ---- persistent tiles -------------------------------------------------
    import contextlib

    _stack = contextlib.ExitStack()
    persist = _stack.enter_context(tc.tile_pool(name="persist", bufs=1))
    qT = persist.tile([128, SEQ], pv, tag="qT")  # [(h,d), s]
    kT = persist.tile([128, SEQ], pv, tag="kT")
    vP = persist.tile([128, TC, 2 * (HD + 1)], pv, tag="vP")  # V' per t-chunk
    xT = [persist.tile([HD + 1, SEQ], pv, tag=f"xT{h}", name=f"xT{h}") for h in range(HPC)]
    wq_sb = persist.tile([128, EC, 128], projdt, tag="wq")
    wk_sb = persist.tile([128, EC, 128], projdt, tag="wk")
    wv_sb = persist.tile([128, EC, 128], projdt, tag="wv")
    wo_sb = persist.tile([64, 2, EMBED], pv, tag="wo")
    id_sb = persist.tile([128, 128], f32, tag="ident")
    bq_sb = persist.tile([128, 1], f32, tag="bq")
    bk_sb = persist.tile([128, 1], f32, tag="bk")
    bv_sb = persist.tile([128, 128], f32, tag="bv")

    w_stage = persist.tile([128, 3, EC, 128], f32, tag="w_stage")
    nc.sync.dma_start(out=w_stage, in_=wqkv.rearrange("w c e d -> e w c d"))
    for i, w_sb in enumerate((wq_sb, wk_sb, wv_sb)):
        if w_sb.dtype == f32:
            nc.sync.dma_start(out=w_sb, in_=wqkv[i].rearrange("c e d -> e c d"))
        else:
            nc.vector.tensor_copy(out=w_sb, in_=w_stage[:, i])
    if pv == f32:
        nc.sync.dma_start(out=wo_sb, in_=wo)
    else:
        wo_f32 = persist.tile([64, 2, EMBED], f32, tag="wo_f32")
        nc.sync.dma_start(out=wo_f32, in_=wo)
        nc.vector.tensor_copy(out=wo_sb, in_=wo_f32)
    nc.sync.dma_start(out=id_sb, in_=ident)
    nc.sync.dma_start(out=bq_sb, in_=bqk[0:1, :].rearrange("a p -> p a"))
    nc.sync.dma_start(out=bk_sb, in_=bqk[1:2, :].rearrange("a p -> p a"))
    bv_bcast = bass.AP(tensor=bv.tensor, offset=bv.offset, ap=[[0, 128], [1, 128]])
    nc.sync.dma_start(out=bv_sb, in_=bv_bcast)
    # ones columns of V' (free positions h*65+64); V overwrites cols 0..63
    ones_sb = persist.tile([128, 1], f32, tag="ones")
    nc.vector.memset(ones_sb, 1.0)
    vP_ones = vP.rearrange("p c (h e) -> p c h e", h=2)[:, :, :, HD : HD + 1]
    ones_b = bass.AP(
        tensor=ones_sb.tensor,
        offset=ones_sb.offset,
        ap=[ones_sb.ap[0], [0, TC], [0, 2], [0, 1]],
    )
    nc.vector.tensor_copy(out=vP_ones, in_=ones_b)

    # ---- phase A: hidden^T + projections ---------------------------------
    with (
        tc.tile_pool(name="hnat", bufs=4) as hnat_p,
        tc.tile_pool(name="hT", bufs=2) as hT_p,
        tc.tile_pool(name="ps_tp", bufs=2, space="PSUM") as tp_ps_p,
        tc.tile_pool(name="ps_proj", bufs=2, space="PSUM") as proj_ps_p,
    ):
        for sb in range(NSUP):  # s-blocks of 512
            hn = [hnat_p.tile([128, EMBED], f32, tag="hn", name=f"hn{sb}_{_j}") for _j in range(JS)]
            for j in range(JS):
                nc.sync.dma_start(out=hn[j], in_=hid[ts(JS * sb + j, 128), :])
            hT = hT_p.tile([128, EC, SUP], projdt, tag="hT")  # [e, chunk, s]
            for c in range(EC):
                tp = tp_ps_p.tile([128, SUP], f32, tag="tp")
                for j in range(JS):
                    nc.tensor.transpose(
                        tp[:, ts(j, 128)], hn[j][:, ts(c, 128)], id_sb
                    )
                nc.vector.tensor_copy(out=hT[:, c, :], in_=tp)
            q_ps = proj_ps_p.tile([128, SUP], f32, tag="q_ps")
            k_ps = proj_ps_p.tile([128, SUP], f32, tag="k_ps")
            v_ps = proj_ps_p.tile([128, SUP], f32, tag="v_ps")
            for c in range(EC):
                nc.tensor.matmul(
                    q_ps, _mm(wq_sb[:, c, :]), _mm(hT[:, c, :]),
                    start=(c == 0), stop=(c == EC - 1),
                )
            for c in range(EC):
                nc.tensor.matmul(
                    k_ps, _mm(wk_sb[:, c, :]), _mm(hT[:, c, :]),
                    start=(c == 0), stop=(c == EC - 1),
                )
            for j in range(JS):  # V natural [t, d], 4 t-tiles per block
                for c in range(EC):
                    nc.tensor.matmul(
                        v_ps[:, ts(j, 128)],
                        _mm(hT[:, c, ts(j, 128)]),
                        _mm(wv_sb[:, c, :]),
                        start=(c == 0), stop=(c == EC - 1),
                        skip_group_check=True,
                    )
            # PSUM -> SBUF with fused bias (and optional bf16 downcast)
            nc.vector.tensor_scalar(
                out=qT[:, ts(sb, SUP)], in0=q_ps,
                scalar1=bq_sb, scalar2=None, op0=AOT.add,
            )
            nc.vector.tensor_scalar(
                out=kT[:, ts(sb, SUP)], in0=k_ps,
                scalar1=bk_sb, scalar2=None, op0=AOT.add,
            )
            for j in range(JS):
                t_idx = JS * sb + j
                dst = vP[:, t_idx, :].rearrange("p (h e) -> p h e", h=2)[:, :, 0:HD]
                nc.vector.tensor_add(
                    out=dst,
                    in0=v_ps[:, ts(j, 128)].rearrange("p (h d) -> p h d", h=2),
                    in1=bv_sb.rearrange("p (h d) -> p h d", h=2),
                )

    # ---- phases B+C: attention + output, per s-super ---------------------
    with (
        tc.tile_pool(name="pT", bufs=2) as pT_p,
        tc.tile_pool(name="dtmp", bufs=2) as dtmp_p,
        tc.tile_pool(name="rD", bufs=2) as rD_p,
        tc.tile_pool(name="anorm", bufs=3) as an_p,
        tc.tile_pool(name="stage", bufs=2) as st_p,
        tc.tile_pool(name="ps_sc", bufs=2, space="PSUM") as sc_ps_p,
        tc.tile_pool(name="ps_at", bufs=1, space="PSUM") as at_ps_p,
        tc.tile_pool(name="ps_wo", bufs=1, space="PSUM") as wo_ps_p,
    ):
        for sup in range(NSUP):
            ssl = ts(sup, SUP)
            at_ps = [
                at_ps_p.tile([HD + 1, SUP], f32, tag=f"at{h}", name=f"at{sup}_{h}") for h in range(HPC)
            ]
            for c in range(TC):
                sc_ps = sc_ps_p.tile([128, 2 * SUP], f32, tag="sc")
                for h in range(HPC):
                    nc.tensor.matmul(
                        sc_ps[:, ts(h, SUP)],
                        _mm(kT[ts(h, HD), ts(c, 128)]),
                        _mm(qT[ts(h, HD), ssl]),
                        start=True, stop=True,
                        tile_position=(h * HD, 0),
                    )
                pT = pT_p.tile([128, 2 * SUP], pv, tag="pT")
                nc.scalar.activation(out=pT, in_=sc_ps, func=Exp)
                for h in range(HPC):
                    nc.tensor.matmul(
                        at_ps[h],
                        _mm(vP[:, c, ts(h, HD + 1)]),
                        _mm(pT[:, ts(h, SUP)]),
                        start=(c == 0), stop=(c == TC - 1),
                    )
            # attn'^T -> SBUF; row 64 is the softmax denominator
            d_tmp = [
                dtmp_p.tile([1, SUP], f32, tag=f"d{h}", name=f"d{sup}_{h}")
                for h in range(HPC)
            ]
            for h in range(HPC):
                nc.vector.tensor_copy(out=xT[h][:, ssl], in_=at_ps[h])
                nc.vector.tensor_copy(
                    out=d_tmp[h], in_=at_ps[h][HD : HD + 1, :]
                )
            # denominators -> s-partitioned layout via tiny PE transposes
            dT_ps = sc_ps_p.tile([128, HPC * JS], f32, tag="sc")
            for h in range(HPC):
                for j in range(JS):
                    nc.tensor.transpose(
                        dT_ps[:, h * JS + j : h * JS + j + 1],
                        d_tmp[h][:, ts(j, 128)],
                        id_sb[0:1, 0:1],
                    )
            rD = rD_p.tile([128, HPC, JS], f32, tag="rD")
            nc.vector.reciprocal(
                out=rD.rearrange("p h j -> p (h j)"), in_=dT_ps
            )
            # Wo per head, then scale by 1/D_h and add heads
            for j in range(JS):
                st_i = JS * sup + j
                stage = st_p.tile([128, EMBED], f32, tag="stage")
                for eh in range(EMBED // SUP):
                    o_ps = [
                        wo_ps_p.tile([128, SUP], f32, tag=f"o{h}", name=f"o{st_i}_{eh}_{h}")
                        for h in range(HPC)
                    ]
                    for h in range(HPC):
                        nc.tensor.matmul(
                            o_ps[h],
                            _mm(xT[h][0:HD, ts(st_i, 128)]),
                            _mm(wo_sb[:, h, ts(eh, SUP)]),
                            start=True, stop=True,
                        )
                    t0 = an_p.tile([128, SUP], f32, tag="t0")
                    t1 = an_p.tile([128, SUP], f32, tag="t1")
                    nc.vector.tensor_scalar_mul(
                        out=t0, in0=o_ps[0], scalar1=rD[:, 0, j : j + 1]
                    )
                    nc.vector.tensor_scalar_mul(
                        out=t1, in0=o_ps[1], scalar1=rD[:, 1, j : j + 1]
                    )
                    nc.vector.tensor_add(
                        out=stage[:, ts(eh, SUP)], in0=t0, in1=t1
                    )
                nc.sync.dma_start(out=outp[ts(st_i, 128), :], in_=stage)

    _stack.close()


def _shards(inputs):
    """Host-side prep: per-core input dicts (head-parallel, Wo row-shard)."""
    hs = np.ascontiguousarray(np.asarray(inputs["hidden_state"], np.float32))
    Wq = np.asarray(inputs["Wq"], np.float32) * 0.125  # fold 1/sqrt(64); exact
    bq = np.asarray(inputs["bq"], np.float32) * 0.125
    Wk = np.asarray(inputs["Wk"], np.float32)
    bk = np.asarray(inputs["bk"], np.float32)
    Wv = np.asarray(inputs["Wv"], np.float32)
    bv = np.asarray(inputs["bv"], np.float32)
    Wo = np.asarray(inputs["Wo"], np.float32)
    ident = np.eye(128, dtype=np.float32)

    in_maps = []
    for c in range(NCORES):
        h0 = HPC * c
        # [H,E,Dh] head-pair -> [E, 2*Dh] -> [EC, 128, 128]
        def _w(W):
            w = np.transpose(W[h0 : h0 + HPC], (1, 0, 2)).reshape(EMBED, 128)
            return np.ascontiguousarray(w.reshape(EC, 128, 128))

        w_qkv = np.stack([_w(Wq), _w(Wk), _w(Wv)])
        b_qk = np.stack(
            [bq[h0 : h0 + HPC].reshape(128), bk[h0 : h0 + HPC].reshape(128)]
        )
        b_v = np.ascontiguousarray(bv[h0 : h0 + HPC].reshape(128))
        # Wo rows for this core's heads: [128, E] -> [64, 2, E] (head-major)
        w_o = np.ascontiguousarray(
            Wo[128 * c : 128 * (c + 1)].reshape(2, 64, EMBED).transpose(1, 0, 2)
        )
        in_maps.append(
            {
                "hidden": hs,
                "w_qkv": np.ascontiguousarray(w_qkv),
                "b_qk": np.ascontiguousarray(b_qk),
                "b_v": b_v,
                "w_o": w_o,
                "ident": ident,
            }
        )
    return in_maps


def kernel(**inputs):
    global LAST
    from concourse import bass_utils

    trace = bool(int(os.environ.get("K_TRACE", "0")))
    if trace:
        _install_ntff_shim()

    key = (MM_DT, PV_DT)
    if key not in _CACHE:
        _CACHE[key] = _build()
    nc = _CACHE[key]

    in_maps = _shards(inputs)
    res = bass_utils.run_bass_kernel_spmd(
        nc, in_maps, core_ids=list(range(NCORES)), trace=trace
    )
    LAST = res

    out = np.zeros((SEQ, EMBED), np.float64)
    for c in range(NCORES):
        out += res.results[c]["out_p"].astype(np.float64)
    out += np.asarray(inputs["bo"], np.float32).astype(np.float64)
    return out.astype(np.float32)


def _install_ntff_shim():
    """antenv.axon_hooks is absent from this image; recreate it so
    run_bass_kernel_spmd(trace=True) can reach the NTFF profiling hook."""
    import types

    if "antenv.axon_hooks" in sys.modules:
        return
    try:
        if "/root/.axon_site" not in sys.path:
            sys.path.insert(0, "/root/.axon_site")
        from trn_agent_boot.trn_boot import _ntff_profile_via_ctypes

        hook = _ntff_profile_via_ctypes("/opt/axon/libaxon_pjrt.so")
    except Exception:
        hook = None
    mod = types.ModuleType("antenv.axon_hooks")
    mod._hook = hook
    mod.get_axon_ntff_profile_hook = lambda: mod._hook
    mod.set_axon_ntff_profile_hook = lambda h: setattr(mod, "_hook", h)
    sys.modules["antenv.axon_hooks"] = mod


# revision 13
# speedup vs baseline: 1.6448x; 1.3375x over previous
"""Multi-head attention (SEQ=4096, EMBED=1024, 16 heads, Dh=64) on 8 TRN2
NeuronCores, head-parallel: 2 heads per core, Wo row-sharded so each core
emits a partial output [SEQ, EMBED]; the host sums the 8 partials (+bo).

Per-core pipeline (one TileContext):
  A) hidden^T e-chunks via PE transposes (fp32 has no DMA transpose);
     projections Q^T,K^T [128(=2 heads x 64 d), S] and V' [t, 2*(64+ones)]
     with the bias fused into the PSUM->SBUF copy. The 1/sqrt(Dh) scale is
     folded into Wq/bq on the host (exact: *0.125).
  B) per s-super(512): scores^T [t, s] as two row-tiled K=64 matmuls (the
     two heads concurrently fill the 128-row array, tile_position (0,0) /
     (64,0)); one Exp per t-chunk over the [128, 2*512] PSUM pair (logits
     are bounded ~|3|, so no max subtraction needed); attn'^T [65, s]
     accumulated in PSUM where the ones column of V' makes row 64 the
     softmax denominator.
  C) denominators -> tiny PE transposes -> DVE reciprocal (s-partitioned);
     Wo per head into separate PSUM banks; DVE scales each head's partial
     by its 1/D and sums; DMA out.
"""

import os
import sys

sys.path.insert(0, "/opt/trn_rl_repo")

import numpy as np

SEQ = 4096
EMBED = 1024
HEADS = 16
HD = 64
NCORES = 8
HPC = HEADS // NCORES  # 2 heads per core
EC = EMBED // 128  # 8 e-chunks
SUP = 512  # s-super size
NSUP = SEQ // SUP  # 8
TC = SEQ // 128  # 32 t-chunks
JS = SUP // 128  # 4 s-tiles per super

# dtype knobs: f32r = fp32 storage, single-pass reduced-precision matmul
MM_DT = os.environ.get("K_MM_DT", "f32r")  # f32r | f32
PV_DT = os.environ.get("K_PV_DT", "f32")  # f32 | bf16 (P^T/V'/x^T/Wo storage)

LAST = None  # BassKernelResults of the most recent run (read by test.py)
_CACHE = {}


def _mm(ap):
    """View an fp32 AP as float32r for single-pass matmuls, if enabled."""
    from concourse import mybir

    if MM_DT == "f32r" and ap.dtype == mybir.dt.float32:
        return ap.bitcast(mybir.dt.float32r)
    return ap


def _build():
    import concourse.bacc as bacc
    import concourse.tile as tile
    from concourse import mybir

    f32 = mybir.dt.float32

    nc = bacc.Bacc("TRN2", debug=False, enable_asserts=False, num_devices=NCORES)

    wqkv = nc.dram_tensor("w_qkv", [3, EC, 128, 128], f32, kind="ExternalInput").ap()
    bqk = nc.dram_tensor("b_qk", [2, 128], f32, kind="ExternalInput").ap()
    bv = nc.dram_tensor("b_v", [128], f32, kind="ExternalInput").ap()
    wo = nc.dram_tensor("w_o", [64, 2, EMBED], f32, kind="ExternalInput").ap()
    ident = nc.dram_tensor("ident", [128, 128], f32, kind="ExternalInput").ap()
    outp = nc.dram_tensor("out_p", [SEQ, EMBED], f32, kind="ExternalOutput").ap()

    if PV_DT == "bf16":
        hidB = nc.dram_tensor(
            "hidden_bf16", [SEQ, EMBED], mybir.dt.bfloat16, kind="ExternalInput"
        ).ap()
        with tile.TileContext(nc) as tc:
            _emit_bf16(tc, mybir, hidB, wqkv, bqk, bv, wo, ident, outp)
    else:
        hid = nc.dram_tensor("hidden", [SEQ, EMBED], f32, kind="ExternalInput").ap()
        with tile.TileContext(nc) as tc:
            _emit(tc, mybir, hid, wqkv, bqk, bv, wo, ident, outp)

    nc.compile()
    return nc


def _emit(tc, mybir, hid, wqkv, bqk, bv, wo, ident, outp):
    import concourse.bass as bass

    nc = tc.nc
    ts = bass.ts
    f32 = mybir.dt.float32
    f32r = mybir.dt.float32r
    mmdt = f32r if MM_DT == "f32r" else f32
    # attention-side storage: bf16 if requested, else the matmul fp32 flavor
    pv = mybir.dt.bfloat16 if PV_DT == "bf16" else mmdt
    projdt = mybir.dt.bfloat16 if PV_DT == "bf16" else mmdt
    Exp = mybir.ActivationFunctionType.Exp
    AOT = mybir.AluOpType

    # ---- persistent tiles -------------------------------------------------
    import contextlib

    _stack = contextlib.ExitStack()
    persist = _stack.enter_context(tc.tile_pool(name="persist", bufs=1))
    qT = persist.tile([128, SEQ], pv, tag="qT")  # [(h,d), s]
    kT = persist.tile([128, SEQ], pv, tag="kT")
    vP = persist.tile([128, TC, 2 * (HD + 1)], pv, tag="vP")  # V' per t-chunk
    xT = [persist.tile([HD + 1, SEQ], pv, tag=f"xT{h}", name=f"xT{h}") for h in range(HPC)]
    wq_sb = persist.tile([128, EC, 128], projdt, tag="wq")
    wk_sb = persist.tile([128, EC, 128], projdt, tag="wk")
    wv_sb = persist.tile([128, EC, 128], projdt, tag="wv")
    wo_sb = persist.tile([64, 2, EMBED], pv, tag="wo")
    id_sb = persist.tile([128, 128], f32, tag="ident")
    bq_sb = persist.tile([128, 1], f32, tag="bq")
    bk_sb = persist.tile([128, 1], f32, tag="bk")
    bv_sb = persist.tile([128, 128], f32, tag="bv")

    w_stage = persist.tile([128, 3, EC, 128], f32, tag="w_stage")
    nc.sync.dma_start(out=w_stage, in_=wqkv.rearrange("w c e d -> e w c d"))
    for i, w_sb in enumerate((wq_sb, wk_sb, wv_sb)):
        if w_sb.dtype == f32:
            nc.sync.dma_start(out=w_sb, in_=wqkv[i].rearrange("c e d -> e c d"))
        else:
            nc.vector.tensor_copy(out=w_sb, in_=w_stage[:, i])
    if pv == f32:
        nc.sync.dma_start(out=wo_sb, in_=wo)
    else:
        wo_f32 = persist.tile([64, 2, EMBED], f32, tag="wo_f32")
        nc.sync.dma_start(out=wo_f32, in_=wo)
        nc.vector.tensor_copy(out=wo_sb, in_=wo_f32)
    nc.sync.dma_start(out=id_sb, in_=ident)
    nc.sync.dma_start(out=bq_sb, in_=bqk[0:1, :].rearrange("a p -> p a"))
    nc.sync.dma_start(out=bk_sb, in_=bqk[1:2, :].rearrange("a p -> p a"))
    bv_bcast = bass.AP(tensor=bv.tensor, offset=bv.offset, ap=[[0, 128], [1, 128]])
    nc.sync.dma_start(out=bv_sb, in_=bv_bcast)
    # ones columns of V' (free positions h*65+64); V overwrites cols 0..63
    ones_sb = persist.tile([128, 1], f32, tag="ones")
    nc.vector.memset(ones_sb, 1.0)
    vP_ones = vP.rearrange("p c (h e) -> p c h e", h=2)[:, :, :, HD : HD + 1]
    ones_b = bass.AP(
        tensor=ones_sb.tensor,
        offset=ones_sb.offset,
        ap=[ones_sb.ap[0], [0, TC], [0, 2], [0, 1]],
    )
    nc.vector.tensor_copy(out=vP_ones, in_=ones_b)

    # ---- phase A: hidden^T + projections ---------------------------------
    with (
        tc.tile_pool(name="hnat", bufs=4) as hnat_p,
        tc.tile_pool(name="hT", bufs=2) as hT_p,
        tc.tile_pool(name="ps_tp", bufs=2, space="PSUM") as tp_ps_p,
        tc.tile_pool(name="ps_proj", bufs=2, space="PSUM") as proj_ps_p,
    ):
        for sb in range(NSUP):  # s-blocks of 512
            hn = [hnat_p.tile([128, EMBED], f32, tag="hn", name=f"hn{sb}_{_j}") for _j in range(JS)]
            for j in range(JS):
                nc.sync.dma_start(out=hn[j], in_=hid[ts(JS * sb + j, 128), :])
            hT = hT_p.tile([128, EC, SUP], projdt, tag="hT")  # [e, chunk, s]
            for c in range(EC):
                tp = tp_ps_p.tile([128, SUP], f32, tag="tp")
                for j in range(JS):
                    nc.tensor.transpose(
                        tp[:, ts(j, 128)], hn[j][:, ts(c, 128)], id_sb
                    )
                nc.vector.tensor_copy(out=hT[:, c, :], in_=tp)
            q_ps = proj_ps_p.tile([128, SUP], f32, tag="q_ps")
            k_ps = proj_ps_p.tile([128, SUP], f32, tag="k_ps")
            v_ps = proj_ps_p.tile([128, SUP], f32, tag="v_ps")
            for c in range(EC):
                nc.tensor.matmul(
                    q_ps, _mm(wq_sb[:, c, :]), _mm(hT[:, c, :]),
                    start=(c == 0), stop=(c == EC - 1),
                )
            for c in range(EC):
                nc.tensor.matmul(
                    k_ps, _mm(wk_sb[:, c, :]), _mm(hT[:, c, :]),
                    start=(c == 0), stop=(c == EC - 1),
                )
            for j in range(JS):  # V natural [t, d], 4 t-tiles per block
                for c in range(EC):
                    nc.tensor.matmul(
                        v_ps[:, ts(j, 128)],
                        _mm(hT[:, c, ts(j, 128)]),
                        _mm(wv_sb[:, c, :]),
                        start=(c == 0), stop=(c == EC - 1),
                        skip_group_check=True,
                    )
            # PSUM -> SBUF with fused bias (and optional bf16 downcast)
            nc.vector.tensor_scalar(
                out=qT[:, ts(sb, SUP)], in0=q_ps,
                scalar1=bq_sb, scalar2=None, op0=AOT.add,
            )
            nc.vector.tensor_scalar(
                out=kT[:, ts(sb, SUP)], in0=k_ps,
                scalar1=bk_sb, scalar2=None, op0=AOT.add,
            )
            for j in range(JS):
                t_idx = JS * sb + j
                dst = vP[:, t_idx, :].rearrange("p (h e) -> p h e", h=2)[:, :, 0:HD]
                nc.vector.tensor_add(
                    out=dst,
                    in0=v_ps[:, ts(j, 128)].rearrange("p (h d) -> p h d", h=2),
                    in1=bv_sb.rearrange("p (h d) -> p h d", h=2),
                )

    # ---- phases B+C: attention + output, per s-super ---------------------
    with (
        tc.tile_pool(name="pT", bufs=2) as pT_p,
        tc.tile_pool(name="dtmp", bufs=2) as dtmp_p,
        tc.tile_pool(name="rD", bufs=2) as rD_p,
        tc.tile_pool(name="anorm", bufs=3) as an_p,
        tc.tile_pool(name="stage", bufs=2) as st_p,
        tc.tile_pool(name="ps_sc", bufs=2, space="PSUM") as sc_ps_p,
        tc.tile_pool(name="ps_at", bufs=1, space="PSUM") as at_ps_p,
        tc.tile_pool(name="ps_wo", bufs=1, space="PSUM") as wo_ps_p,
    ):
        for sup in range(NSUP):
            ssl = ts(sup, SUP)
            at_ps = [
                at_ps_p.tile([HD + 1, SUP], f32, tag=f"at{h}", name=f"at{sup}_{h}") for h in range(HPC)
            ]
            for c in range(TC):
                sc_ps = sc_ps_p.tile([128, 2 * SUP], f32, tag="sc")
                for h in range(HPC):
                    nc.tensor.matmul(
                        sc_ps[:, ts(h, SUP)],
                        _mm(kT[ts(h, HD), ts(c, 128)]),
                        _mm(qT[ts(h, HD), ssl]),
                        start=True, stop=True,
                        tile_position=(h * HD, 0),
                    )
                pT = pT_p.tile([128, 2 * SUP], pv, tag="pT")
                nc.scalar.activation(out=pT, in_=sc_ps, func=Exp)
                for h in range(HPC):
                    nc.tensor.matmul(
                        at_ps[h],
                        _mm(vP[:, c, ts(h, HD + 1)]),
                        _mm(pT[:, ts(h, SUP)]),
                        start=(c == 0), stop=(c == TC - 1),
                    )
            # attn'^T -> SBUF; row 64 is the softmax denominator
            d_tmp = [
                dtmp_p.tile([1, SUP], f32, tag=f"d{h}", name=f"d{sup}_{h}")
                for h in range(HPC)
            ]
            for h in range(HPC):
                nc.vector.tensor_copy(out=xT[h][:, ssl], in_=at_ps[h])
                nc.vector.tensor_copy(
                    out=d_tmp[h], in_=at_ps[h][HD : HD + 1, :]
                )
            # denominators -> s-partitioned layout via tiny PE transposes
            dT_ps = sc_ps_p.tile([128, HPC * JS], f32, tag="sc")
            for h in range(HPC):
                for j in range(JS):
                    nc.tensor.transpose(
                        dT_ps[:, h * JS + j : h * JS + j + 1],
                        d_tmp[h][:, ts(j, 128)],
                        id_sb[0:1, 0:1],
                    )
            rD = rD_p.tile([128, HPC, JS], f32, tag="rD")
            nc.vector.reciprocal(
                out=rD.rearrange("p h j -> p (h j)"), in_=dT_ps
            )
            # Wo per head, then scale by 1/D_h and add heads
            for j in range(JS):
                st_i = JS * sup + j
                stage = st_p.tile([128, EMBED], f32, tag="stage")
                for eh in range(EMBED // SUP):
                    o_ps = [
                        wo_ps_p.tile([128, SUP], f32, tag=f"o{h}", name=f"o{st_i}_{eh}_{h}")
                        for h in range(HPC)
                    ]
                    for h in range(HPC):
                        nc.tensor.matmul(
                            o_ps[h],
                            _mm(xT[h][0:HD, ts(st_i, 128)]),
                            _mm(wo_sb[:, h, ts(eh, SUP)]),
                            start=True, stop=True,
                        )
                    t0 = an_p.tile([128, SUP], f32, tag="t0")
                    t1 = an_p.tile([128, SUP], f32, tag="t1")
                    nc.vector.tensor_scalar_mul(
                        out=t0, in0=o_ps[0], scalar1=rD[:, 0, j : j + 1]
                    )
                    nc.vector.tensor_scalar_mul(
                        out=t1, in0=o_ps[1], scalar1=rD[:, 1, j : j + 1]
                    )
                    nc.vector.tensor_add(
                        out=stage[:, ts(eh, SUP)], in0=t0, in1=t1
                    )
                nc.sync.dma_start(out=outp[ts(st_i, 128), :], in_=stage)

    _stack.close()


def _emit_bf16(tc, mybir, hidB, wqkv, bqk, bv, wo, ident, outp):
    """bf16 path: DMA-xbar-transposed hidden, A/B interleave, trailing C."""
    import concourse.bass as bass

    nc = tc.nc
    ts = bass.ts
    f32 = mybir.dt.float32
    bf16 = mybir.dt.bfloat16
    Exp = mybir.ActivationFunctionType.Exp
    AOT = mybir.AluOpType

    import contextlib

    st_ = contextlib.ExitStack()
    persist = st_.enter_context(tc.tile_pool(name="persist", bufs=1))
    qT = persist.tile([128, SEQ], bf16, tag="qT")
    kT = persist.tile([128, SEQ], bf16, tag="kT")
    vP = persist.tile([128, TC, 2 * (HD + 1)], bf16, tag="vP")
    xT = [persist.tile([HD + 1, SEQ], bf16, tag=f"xT{h}", name=f"xT{h}") for h in range(HPC)]
    hTa = persist.tile([128, EC, SEQ], bf16, tag="hTa")  # hidden^T, all chunks
    wq_sb = persist.tile([128, EC, 128], bf16, tag="wq")
    wk_sb = persist.tile([128, EC, 128], bf16, tag="wk")
    wv_sb = persist.tile([128, EC, 128], bf16, tag="wv")
    wo_sb = persist.tile([64, 2, EMBED], bf16, tag="wo")
    id_sb = persist.tile([128, 128], f32, tag="ident")
    bq_sb = persist.tile([128, 1], f32, tag="bq")
    bk_sb = persist.tile([128, 1], f32, tag="bk")
    bv_sb = persist.tile([128, 128], f32, tag="bv")

    with tc.tile_pool(name="wstage", bufs=1) as wst_p:
        w_stage = wst_p.tile([128, 3, EC, 128], f32, tag="wst")
        nc.sync.dma_start(out=w_stage, in_=wqkv.rearrange("w c e d -> e w c d"))
        for i, w_sb in enumerate((wq_sb, wk_sb, wv_sb)):
            nc.vector.tensor_copy(out=w_sb, in_=w_stage[:, i])
        wo_f32 = wst_p.tile([64, 2, EMBED], f32, tag="wof")
        nc.sync.dma_start(out=wo_f32, in_=wo)
        nc.vector.tensor_copy(out=wo_sb, in_=wo_f32)
    nc.sync.dma_start(out=id_sb, in_=ident)
    nc.sync.dma_start(out=bq_sb, in_=bqk[0:1, :].rearrange("a p -> p a"))
    nc.sync.dma_start(out=bk_sb, in_=bqk[1:2, :].rearrange("a p -> p a"))
    bv_bcast = bass.AP(tensor=bv.tensor, offset=bv.offset, ap=[[0, 128], [1, 128]])
    nc.sync.dma_start(out=bv_sb, in_=bv_bcast)
    ones_sb = persist.tile([128, 1], f32, tag="ones")
    nc.vector.memset(ones_sb, 1.0)
    vP_ones = vP.rearrange("p c (h e) -> p c h e", h=2)[:, :, :, HD : HD + 1]
    ones_b = bass.AP(
        tensor=ones_sb.tensor, offset=ones_sb.offset,
        ap=[ones_sb.ap[0], [0, TC], [0, 2], [0, 1]],
    )
    nc.vector.tensor_copy(out=vP_ones, in_=ones_b)
    # hidden^T via DMA xbar transpose (2-byte dtype)
    for c in range(EC):
        nc.sync.dma_start(
            out=hTa[:, c, :], in_=hidB[:, ts(c, 128)], transpose=True
        )

    pT_p = st_.enter_context(tc.tile_pool(name="pT", bufs=2))
    dtmp_p = st_.enter_context(tc.tile_pool(name="dtmp", bufs=2))
    rD_p = st_.enter_context(tc.tile_pool(name="rD", bufs=2))
    an_p = st_.enter_context(tc.tile_pool(name="anorm", bufs=3))
    stg_p = st_.enter_context(tc.tile_pool(name="stage", bufs=2))
    sc_ps_p = st_.enter_context(tc.tile_pool(name="ps_sc", bufs=2, space="PSUM"))
    at_ps_p = st_.enter_context(tc.tile_pool(name="ps_at", bufs=1, space="PSUM"))
    aux_ps_p = st_.enter_context(tc.tile_pool(name="ps_aux", bufs=2, space="PSUM"))

    rd_of = {}
    at_of = {}
    d_of = {}

    def q_proj(sup):
        q_ps = aux_ps_p.tile([128, SUP], f32, tag="aux", name=f"q_ps{sup}")
        for c in range(EC):
            nc.tensor.matmul(
                q_ps, wq_sb[:, c, :], hTa[:, c, ts(sup, SUP)],
                start=(c == 0), stop=(c == EC - 1),
            )
        nc.vector.tensor_scalar(
            out=qT[:, ts(sup, SUP)], in0=q_ps,
            scalar1=bq_sb, scalar2=None, op0=AOT.add,
        )

    def kv_block(b):
        k_ps = aux_ps_p.tile([128, SUP], f32, tag="aux", name=f"k_ps{b}")
        for c in range(EC):
            nc.tensor.matmul(
                k_ps, wk_sb[:, c, :], hTa[:, c, ts(b, SUP)],
                start=(c == 0), stop=(c == EC - 1),
            )
        nc.vector.tensor_scalar(
            out=kT[:, ts(b, SUP)], in0=k_ps,
            scalar1=bk_sb, scalar2=None, op0=AOT.add,
        )
        v_ps = aux_ps_p.tile([128, SUP], f32, tag="aux", name=f"v_ps{b}")
        for j in range(JS):
            for c in range(EC):
                nc.tensor.matmul(
                    v_ps[:, ts(j, 128)],
                    hTa[:, c, ts(JS * b + j, 128)],
                    wv_sb[:, c, :],
                    start=(c == 0), stop=(c == EC - 1),
                    skip_group_check=True,
                )
        for j in range(JS):
            t_idx = JS * b + j
            dst = vP[:, t_idx, :].rearrange("p (h e) -> p h e", h=2)[:, :, 0:HD]
            nc.vector.tensor_add(
                out=dst,
                in0=v_ps[:, ts(j, 128)].rearrange("p (h d) -> p h d", h=2),
                in1=bv_sb.rearrange("p (h d) -> p h d", h=2),
            )

    def b_chunk(sup, c):
        sc_ps = sc_ps_p.tile([128, 2 * SUP], f32, tag="sc", name=f"sc{sup}_{c}")
        for h in range(HPC):
            nc.tensor.matmul(
                sc_ps[:, ts(h, SUP)],
                kT[ts(h, HD), ts(c, 128)],
                qT[ts(h, HD), ts(sup, SUP)],
                start=True, stop=True,
                tile_position=(h * HD, 0),
            )
        pT = pT_p.tile([128, 2 * SUP], bf16, tag="pT", name=f"pT{sup}_{c}")
        nc.scalar.activation(out=pT, in_=sc_ps, func=Exp)
        for h in range(HPC):
            nc.tensor.matmul(
                at_of[sup][h],
                vP[:, c, ts(h, HD + 1)],
                pT[:, ts(h, SUP)],
                start=(c == 0), stop=(c == TC - 1),
            )

    def drain(sup):
        dts = [
            dtmp_p.tile([1, SUP], f32, tag=f"d{h}", name=f"d{sup}_{h}")
            for h in range(HPC)
        ]
        d_of[sup] = dts
        for h in range(HPC):
            nc.vector.tensor_copy(out=xT[h][:, ts(sup, SUP)], in_=at_of[sup][h])
            nc.vector.tensor_copy(out=dts[h], in_=at_of[sup][h][HD : HD + 1, :])

    def c_head(sup):
        # denominators -> s-partitioned reciprocals (reads xT row 64)
        dT_ps = sc_ps_p.tile([128, HPC * JS], f32, tag="sc", name=f"dT{sup}")
        for h in range(HPC):
            for j in range(JS):
                nc.tensor.transpose(
                    dT_ps[:, h * JS + j : h * JS + j + 1],
                    d_of[sup][h][:, ts(j, 128)],
                    id_sb[0:1, 0:1],
                )
        rD = rD_p.tile([128, HPC, JS], f32, tag="rD", name=f"rD{sup}")
        nc.vector.reciprocal(out=rD.rearrange("p h j -> p (h j)"), in_=dT_ps)
        rd_of[sup] = rD

    def c_unit(sup, j, eh, stage):
        st_i = JS * sup + j
        rD = rd_of[sup]
        o_ps = [
            aux_ps_p.tile([128, SUP], f32, tag="aux", name=f"o{st_i}_{eh}_{h}")
            for h in range(HPC)
        ]
        for h in range(HPC):
            nc.tensor.matmul(
                o_ps[h],
                xT[h][0:HD, ts(st_i, 128)],
                wo_sb[:, h, ts(eh, SUP)],
                start=True, stop=True,
            )
        t0 = an_p.tile([128, SUP], f32, tag="t0", name=f"t0_{st_i}_{eh}")
        t1 = an_p.tile([128, SUP], f32, tag="t1", name=f"t1_{st_i}_{eh}")
        nc.vector.tensor_scalar_mul(out=t0, in0=o_ps[0], scalar1=rD[:, 0, j : j + 1])
        nc.vector.tensor_scalar_mul(out=t1, in0=o_ps[1], scalar1=rD[:, 1, j : j + 1])
        nc.vector.tensor_add(out=stage[:, ts(eh, SUP)], in0=t0, in1=t1)
        if eh == EMBED // SUP - 1:
            nc.sync.dma_start(out=outp[ts(st_i, 128), :], in_=stage)

    def c_tail(sup, slot):
        # slot 0: head (D/recip); slots 1..8: the 8 (j, eh) units
        if slot == 0:
            c_head(sup)
        else:
            u = slot - 1
            j, eh = divmod(u, EMBED // SUP)
            if eh == 0:
                stage_tiles[sup % 2][j] = stg_p.tile(
                    [128, EMBED], f32, tag="stage", name=f"stage{sup}_{j}"
                )
            c_unit(sup, j, eh, stage_tiles[sup % 2][j])

    stage_tiles = [[None] * JS, [None] * JS]

    # ---- phase A interleaved with super 0 --------------------------------
    at_of[0] = [
        at_ps_p.tile([HD + 1, SUP], f32, tag=f"at{h}", name=f"at0_{h}")
        for h in range(HPC)
    ]
    for b in range(NSUP):
        kv_block(b)
        if b == 0:
            q_proj(0)
        for c in range(JS * b, JS * b + JS):
            b_chunk(0, c)

    # ---- supers 1..7 with trailing C(sup-1) ------------------------------
    for sup in range(1, NSUP):
        q_proj(sup)
        drain(sup - 1)
        at_of[sup] = [
            at_ps_p.tile([HD + 1, SUP], f32, tag=f"at{h}", name=f"at{sup}_{h}")
            for h in range(HPC)
        ]
        slot = 0
        for c in range(TC):
            b_chunk(sup, c)
            if c % 3 == 2 and slot < 9:
                c_tail(sup - 1, slot)
                slot += 1
        while slot < 9:
            c_tail(sup - 1, slot)
            slot += 1
    drain(NSUP - 1)
    for slot in range(9):
        c_tail(NSUP - 1, slot)

    st_.close()


def _shards(inputs):
    """Host-side prep: per-core input dicts (head-parallel, Wo row-shard)."""
    hs = np.ascontiguousarray(np.asarray(inputs["hidden_state"], np.float32))
    Wq = np.asarray(inputs["Wq"], np.float32) * 0.125  # fold 1/sqrt(64); exact
    bq = np.asarray(inputs["bq"], np.float32) * 0.125
    Wk = np.asarray(inputs["Wk"], np.float32)
    bk = np.asarray(inputs["bk"], np.float32)
    Wv = np.asarray(inputs["Wv"], np.float32)
    bv = np.asarray(inputs["bv"], np.float32)
    Wo = np.asarray(inputs["Wo"], np.float32)
    ident = np.eye(128, dtype=np.float32)
    hs_bf16 = None
    if PV_DT == "bf16":
        import ml_dtypes

        hs_bf16 = np.ascontiguousarray(hs.astype(ml_dtypes.bfloat16))

    in_maps = []
    for c in range(NCORES):
        h0 = HPC * c
        # [H,E,Dh] head-pair -> [E, 2*Dh] -> [EC, 128, 128]
        def _w(W):
            w = np.transpose(W[h0 : h0 + HPC], (1, 0, 2)).reshape(EMBED, 128)
            return np.ascontiguousarray(w.reshape(EC, 128, 128))

        w_qkv = np.stack([_w(Wq), _w(Wk), _w(Wv)])
        b_qk = np.stack(
            [bq[h0 : h0 + HPC].reshape(128), bk[h0 : h0 + HPC].reshape(128)]
        )
        b_v = np.ascontiguousarray(bv[h0 : h0 + HPC].reshape(128))
        # Wo rows for this core's heads: [128, E] -> [64, 2, E] (head-major)
        w_o = np.ascontiguousarray(
            Wo[128 * c : 128 * (c + 1)].reshape(2, 64, EMBED).transpose(1, 0, 2)
        )
        im = (
            {"hidden_bf16": hs_bf16} if PV_DT == "bf16" else {"hidden": hs}
        )
        in_maps.append(
            {
                **im,
                "w_qkv": np.ascontiguousarray(w_qkv),
                "b_qk": np.ascontiguousarray(b_qk),
                "b_v": b_v,
                "w_o": w_o,
                "ident": ident,
            }
        )
    return in_maps


def kernel(**inputs):
    global LAST
    from concourse import bass_utils

    trace = bool(int(os.environ.get("K_TRACE", "0")))
    if trace:
        _install_ntff_shim()

    key = (MM_DT, PV_DT)
    if key not in _CACHE:
        _CACHE[key] = _build()
    nc = _CACHE[key]

    in_maps = _shards(inputs)
    res = bass_utils.run_bass_kernel_spmd(
        nc, in_maps, core_ids=list(range(NCORES)), trace=trace
    )
    LAST = res

    out = np.zeros((SEQ, EMBED), np.float64)
    for c in range(NCORES):
        out += res.results[c]["out_p"].astype(np.float64)
    out += np.asarray(inputs["bo"], np.float32).astype(np.float64)
    return out.astype(np.float32)


def _install_ntff_shim():
    """antenv.axon_hooks is absent from this image; recreate it so
    run_bass_kernel_spmd(trace=True) can reach the NTFF profiling hook."""
    import types

    if "antenv.axon_hooks" in sys.modules:
        return
    try:
        if "/root/.axon_site" not in sys.path:
            sys.path.insert(0, "/root/.axon_site")
        from trn_agent_boot.trn_boot import _ntff_profile_via_ctypes

        hook = _ntff_profile_via_ctypes("/opt/axon/libaxon_pjrt.so")
    except Exception:
        hook = None
    mod = types.ModuleType("antenv.axon_hooks")
    mod._hook = hook
    mod.get_axon_ntff_profile_hook = lambda: mod._hook
    mod.set_axon_ntff_profile_hook = lambda h: setattr(mod, "_hook", h)
    sys.modules["antenv.axon_hooks"] = mod


# revision 14
# speedup vs baseline: 1.6502x; 1.0033x over previous
"""Multi-head attention (SEQ=4096, EMBED=1024, 16 heads, Dh=64) on 8 TRN2
NeuronCores, head-parallel: 2 heads per core, Wo row-sharded so each core
emits a partial output [SEQ, EMBED]; the host sums the 8 partials (+bo).

Per-core pipeline (one TileContext):
  A) hidden^T e-chunks via PE transposes (fp32 has no DMA transpose);
     projections Q^T,K^T [128(=2 heads x 64 d), S] and V' [t, 2*(64+ones)]
     with the bias fused into the PSUM->SBUF copy. The 1/sqrt(Dh) scale is
     folded into Wq/bq on the host (exact: *0.125).
  B) per s-super(512): scores^T [t, s] as two row-tiled K=64 matmuls (the
     two heads concurrently fill the 128-row array, tile_position (0,0) /
     (64,0)); one Exp per t-chunk over the [128, 2*512] PSUM pair (logits
     are bounded ~|3|, so no max subtraction needed); attn'^T [65, s]
     accumulated in PSUM where the ones column of V' makes row 64 the
     softmax denominator.
  C) denominators -> tiny PE transposes -> DVE reciprocal (s-partitioned);
     Wo per head into separate PSUM banks; DVE scales each head's partial
     by its 1/D and sums; DMA out.
"""

import os
import sys

sys.path.insert(0, "/opt/trn_rl_repo")

import numpy as np

SEQ = 4096
EMBED = 1024
HEADS = 16
HD = 64
NCORES = 8
HPC = HEADS // NCORES  # 2 heads per core
EC = EMBED // 128  # 8 e-chunks
SUP = 512  # s-super size
NSUP = SEQ // SUP  # 8
TC = SEQ // 128  # 32 t-chunks
JS = SUP // 128  # 4 s-tiles per super

# dtype knobs: f32r = fp32 storage, single-pass reduced-precision matmul
MM_DT = os.environ.get("K_MM_DT", "f32r")  # f32r | f32
PV_DT = os.environ.get("K_PV_DT", "f32")  # f32 | bf16 (P^T/V'/x^T/Wo storage)

LAST = None  # BassKernelResults of the most recent run (read by test.py)
_CACHE = {}


def _mm(ap):
    """View an fp32 AP as float32r for single-pass matmuls, if enabled."""
    from concourse import mybir

    if MM_DT == "f32r" and ap.dtype == mybir.dt.float32:
        return ap.bitcast(mybir.dt.float32r)
    return ap


def _build():
    import concourse.bacc as bacc
    import concourse.tile as tile
    from concourse import mybir

    f32 = mybir.dt.float32

    nc = bacc.Bacc("TRN2", debug=False, enable_asserts=False, num_devices=NCORES)

    wqkv = nc.dram_tensor("w_qkv", [3, EC, 128, 128], f32, kind="ExternalInput").ap()
    bqk = nc.dram_tensor("b_qk", [2, 128], f32, kind="ExternalInput").ap()
    bv = nc.dram_tensor("b_v", [128], f32, kind="ExternalInput").ap()
    wo = nc.dram_tensor("w_o", [64, 2, EMBED], f32, kind="ExternalInput").ap()
    ident = nc.dram_tensor("ident", [128, 128], f32, kind="ExternalInput").ap()
    outp = nc.dram_tensor("out_p", [SEQ, EMBED], f32, kind="ExternalOutput").ap()

    if PV_DT == "bf16":
        hidB = nc.dram_tensor(
            "hidden_bf16", [SEQ, EMBED], mybir.dt.bfloat16, kind="ExternalInput"
        ).ap()
        with tile.TileContext(nc) as tc:
            _emit_bf16(tc, mybir, hidB, wqkv, bqk, bv, wo, ident, outp)
    else:
        hid = nc.dram_tensor("hidden", [SEQ, EMBED], f32, kind="ExternalInput").ap()
        with tile.TileContext(nc) as tc:
            _emit(tc, mybir, hid, wqkv, bqk, bv, wo, ident, outp)

    nc.compile()
    return nc


def _emit(tc, mybir, hid, wqkv, bqk, bv, wo, ident, outp):
    import concourse.bass as bass

    nc = tc.nc
    ts = bass.ts
    f32 = mybir.dt.float32
    f32r = mybir.dt.float32r
    mmdt = f32r if MM_DT == "f32r" else f32
    # attention-side storage: bf16 if requested, else the matmul fp32 flavor
    pv = mybir.dt.bfloat16 if PV_DT == "bf16" else mmdt
    projdt = mybir.dt.bfloat16 if PV_DT == "bf16" else mmdt
    Exp = mybir.ActivationFunctionType.Exp
    AOT = mybir.AluOpType

    # ---- persistent tiles -------------------------------------------------
    import contextlib

    _stack = contextlib.ExitStack()
    persist = _stack.enter_context(tc.tile_pool(name="persist", bufs=1))
    qT = persist.tile([128, SEQ], pv, tag="qT")  # [(h,d), s]
    kT = persist.tile([128, SEQ], pv, tag="kT")
    vP = persist.tile([128, TC, 2 * (HD + 1)], pv, tag="vP")  # V' per t-chunk
    xT = [persist.tile([HD + 1, SEQ], pv, tag=f"xT{h}", name=f"xT{h}") for h in range(HPC)]
    wq_sb = persist.tile([128, EC, 128], projdt, tag="wq")
    wk_sb = persist.tile([128, EC, 128], projdt, tag="wk")
    wv_sb = persist.tile([128, EC, 128], projdt, tag="wv")
    wo_sb = persist.tile([64, 2, EMBED], pv, tag="wo")
    id_sb = persist.tile([128, 128], f32, tag="ident")
    bq_sb = persist.tile([128, 1], f32, tag="bq")
    bk_sb = persist.tile([128, 1], f32, tag="bk")
    bv_sb = persist.tile([128, 128], f32, tag="bv")

    w_stage = persist.tile([128, 3, EC, 128], f32, tag="w_stage")
    nc.sync.dma_start(out=w_stage, in_=wqkv.rearrange("w c e d -> e w c d"))
    for i, w_sb in enumerate((wq_sb, wk_sb, wv_sb)):
        if w_sb.dtype == f32:
            nc.sync.dma_start(out=w_sb, in_=wqkv[i].rearrange("c e d -> e c d"))
        else:
            nc.vector.tensor_copy(out=w_sb, in_=w_stage[:, i])
    if pv == f32:
        nc.sync.dma_start(out=wo_sb, in_=wo)
    else:
        wo_f32 = persist.tile([64, 2, EMBED], f32, tag="wo_f32")
        nc.sync.dma_start(out=wo_f32, in_=wo)
        nc.vector.tensor_copy(out=wo_sb, in_=wo_f32)
    nc.sync.dma_start(out=id_sb, in_=ident)
    nc.sync.dma_start(out=bq_sb, in_=bqk[0:1, :].rearrange("a p -> p a"))
    nc.sync.dma_start(out=bk_sb, in_=bqk[1:2, :].rearrange("a p -> p a"))
    bv_bcast = bass.AP(tensor=bv.tensor, offset=bv.offset, ap=[[0, 128], [1, 128]])
    nc.sync.dma_start(out=bv_sb, in_=bv_bcast)
    # ones columns of V' (free positions h*65+64); V overwrites cols 0..63
    ones_sb = persist.tile([128, 1], f32, tag="ones")
    nc.vector.memset(ones_sb, 1.0)
    vP_ones = vP.rearrange("p c (h e) -> p c h e", h=2)[:, :, :, HD : HD + 1]
    ones_b = bass.AP(
        tensor=ones_sb.tensor,
        offset=ones_sb.offset,
        ap=[ones_sb.ap[0], [0, TC], [0, 2], [0, 1]],
    )
    nc.vector.tensor_copy(out=vP_ones, in_=ones_b)

    # ---- phase A: hidden^T + projections ---------------------------------
    with (
        tc.tile_pool(name="hnat", bufs=4) as hnat_p,
        tc.tile_pool(name="hT", bufs=2) as hT_p,
        tc.tile_pool(name="ps_tp", bufs=2, space="PSUM") as tp_ps_p,
        tc.tile_pool(name="ps_proj", bufs=2, space="PSUM") as proj_ps_p,
    ):
        for sb in range(NSUP):  # s-blocks of 512
            hn = [hnat_p.tile([128, EMBED], f32, tag="hn", name=f"hn{sb}_{_j}") for _j in range(JS)]
            for j in range(JS):
                nc.sync.dma_start(out=hn[j], in_=hid[ts(JS * sb + j, 128), :])
            hT = hT_p.tile([128, EC, SUP], projdt, tag="hT")  # [e, chunk, s]
            for c in range(EC):
                tp = tp_ps_p.tile([128, SUP], f32, tag="tp")
                for j in range(JS):
                    nc.tensor.transpose(
                        tp[:, ts(j, 128)], hn[j][:, ts(c, 128)], id_sb
                    )
                nc.vector.tensor_copy(out=hT[:, c, :], in_=tp)
            q_ps = proj_ps_p.tile([128, SUP], f32, tag="q_ps")
            k_ps = proj_ps_p.tile([128, SUP], f32, tag="k_ps")
            v_ps = proj_ps_p.tile([128, SUP], f32, tag="v_ps")
            for c in range(EC):
                nc.tensor.matmul(
                    q_ps, _mm(wq_sb[:, c, :]), _mm(hT[:, c, :]),
                    start=(c == 0), stop=(c == EC - 1),
                )
            for c in range(EC):
                nc.tensor.matmul(
                    k_ps, _mm(wk_sb[:, c, :]), _mm(hT[:, c, :]),
                    start=(c == 0), stop=(c == EC - 1),
                )
            for j in range(JS):  # V natural [t, d], 4 t-tiles per block
                for c in range(EC):
                    nc.tensor.matmul(
                        v_ps[:, ts(j, 128)],
                        _mm(hT[:, c, ts(j, 128)]),
                        _mm(wv_sb[:, c, :]),
                        start=(c == 0), stop=(c == EC - 1),
                        skip_group_check=True,
                    )
            # PSUM -> SBUF with fused bias (and optional bf16 downcast)
            nc.vector.tensor_scalar(
                out=qT[:, ts(sb, SUP)], in0=q_ps,
                scalar1=bq_sb, scalar2=None, op0=AOT.add,
            )
            nc.vector.tensor_scalar(
                out=kT[:, ts(sb, SUP)], in0=k_ps,
                scalar1=bk_sb, scalar2=None, op0=AOT.add,
            )
            for j in range(JS):
                t_idx = JS * sb + j
                dst = vP[:, t_idx, :].rearrange("p (h e) -> p h e", h=2)[:, :, 0:HD]
                nc.vector.tensor_add(
                    out=dst,
                    in0=v_ps[:, ts(j, 128)].rearrange("p (h d) -> p h d", h=2),
                    in1=bv_sb.rearrange("p (h d) -> p h d", h=2),
                )

    # ---- phases B+C: attention + output, per s-super ---------------------
    with (
        tc.tile_pool(name="pT", bufs=2) as pT_p,
        tc.tile_pool(name="dtmp", bufs=2) as dtmp_p,
        tc.tile_pool(name="rD", bufs=2) as rD_p,
        tc.tile_pool(name="anorm", bufs=3) as an_p,
        tc.tile_pool(name="stage", bufs=2) as st_p,
        tc.tile_pool(name="ps_sc", bufs=2, space="PSUM") as sc_ps_p,
        tc.tile_pool(name="ps_at", bufs=1, space="PSUM") as at_ps_p,
        tc.tile_pool(name="ps_wo", bufs=1, space="PSUM") as wo_ps_p,
    ):
        for sup in range(NSUP):
            ssl = ts(sup, SUP)
            at_ps = [
                at_ps_p.tile([HD + 1, SUP], f32, tag=f"at{h}", name=f"at{sup}_{h}") for h in range(HPC)
            ]
            for c in range(TC):
                sc_ps = sc_ps_p.tile([128, 2 * SUP], f32, tag="sc")
                for h in range(HPC):
                    nc.tensor.matmul(
                        sc_ps[:, ts(h, SUP)],
                        _mm(kT[ts(h, HD), ts(c, 128)]),
                        _mm(qT[ts(h, HD), ssl]),
                        start=True, stop=True,
                        tile_position=(h * HD, 0),
                    )
                pT = pT_p.tile([128, 2 * SUP], pv, tag="pT")
                nc.scalar.activation(out=pT, in_=sc_ps, func=Exp)
                for h in range(HPC):
                    nc.tensor.matmul(
                        at_ps[h],
                        _mm(vP[:, c, ts(h, HD + 1)]),
                        _mm(pT[:, ts(h, SUP)]),
                        start=(c == 0), stop=(c == TC - 1),
                    )
            # attn'^T -> SBUF; row 64 is the softmax denominator
            d_tmp = [
                dtmp_p.tile([1, SUP], f32, tag=f"d{h}", name=f"d{sup}_{h}")
                for h in range(HPC)
            ]
            for h in range(HPC):
                nc.vector.tensor_copy(out=xT[h][:, ssl], in_=at_ps[h])
                nc.vector.tensor_copy(
                    out=d_tmp[h], in_=at_ps[h][HD : HD + 1, :]
                )
            # denominators -> s-partitioned layout via tiny PE transposes
            dT_ps = sc_ps_p.tile([128, HPC * JS], f32, tag="sc")
            for h in range(HPC):
                for j in range(JS):
                    nc.tensor.transpose(
                        dT_ps[:, h * JS + j : h * JS + j + 1],
                        d_tmp[h][:, ts(j, 128)],
                        id_sb[0:1, 0:1],
                    )
            rD = rD_p.tile([128, HPC, JS], f32, tag="rD")
            nc.vector.reciprocal(
                out=rD.rearrange("p h j -> p (h j)"), in_=dT_ps
            )
            # Wo per head, then scale by 1/D_h and add heads
            for j in range(JS):
                st_i = JS * sup + j
                stage = st_p.tile([128, EMBED], f32, tag="stage")
                for eh in range(EMBED // SUP):
                    o_ps = [
                        wo_ps_p.tile([128, SUP], f32, tag=f"o{h}", name=f"o{st_i}_{eh}_{h}")
                        for h in range(HPC)
                    ]
                    for h in range(HPC):
                        nc.tensor.matmul(
                            o_ps[h],
                            _mm(xT[h][0:HD, ts(st_i, 128)]),
                            _mm(wo_sb[:, h, ts(eh, SUP)]),
                            start=True, stop=True,
                        )
                    t0 = an_p.tile([128, SUP], f32, tag="t0")
                    t1 = an_p.tile([128, SUP], f32, tag="t1")
                    nc.vector.tensor_scalar_mul(
                        out=t0, in0=o_ps[0], scalar1=rD[:, 0, j : j + 1]
                    )
                    nc.vector.tensor_scalar_mul(
                        out=t1, in0=o_ps[1], scalar1=rD[:, 1, j : j + 1]
                    )
                    nc.vector.tensor_add(
                        out=stage[:, ts(eh, SUP)], in0=t0, in1=t1
                    )
                nc.sync.dma_start(out=outp[ts(st_i, 128), :], in_=stage)

    _stack.close()


def _emit_bf16(tc, mybir, hidB, wqkv, bqk, bv, wo, ident, outp):
    """bf16 path: DMA-xbar-transposed hidden, A/B interleave, trailing C."""
    import concourse.bass as bass

    nc = tc.nc
    ts = bass.ts
    f32 = mybir.dt.float32
    bf16 = mybir.dt.bfloat16
    Exp = mybir.ActivationFunctionType.Exp
    AOT = mybir.AluOpType

    import contextlib

    st_ = contextlib.ExitStack()
    persist = st_.enter_context(tc.tile_pool(name="persist", bufs=1))
    qT = persist.tile([128, SEQ], bf16, tag="qT")
    kT = persist.tile([128, SEQ], bf16, tag="kT")
    vP = persist.tile([128, TC, 2 * (HD + 1)], bf16, tag="vP")
    xT = [persist.tile([HD + 1, SEQ], bf16, tag=f"xT{h}", name=f"xT{h}") for h in range(HPC)]
    hTa = persist.tile([128, EC, SEQ], bf16, tag="hTa")  # hidden^T, all chunks
    wq_sb = persist.tile([128, EC, 128], bf16, tag="wq")
    wk_sb = persist.tile([128, EC, 128], bf16, tag="wk")
    wv_sb = persist.tile([128, EC, 128], bf16, tag="wv")
    wo_sb = persist.tile([64, 2, EMBED], bf16, tag="wo")
    id_sb = persist.tile([128, 128], f32, tag="ident")
    bq_sb = persist.tile([128, 1], f32, tag="bq")
    bk_sb = persist.tile([128, 1], f32, tag="bk")
    bv_sb = persist.tile([128, 128], f32, tag="bv")

    with tc.tile_pool(name="wstage", bufs=1) as wst_p:
        w_stage = wst_p.tile([128, 3, EC, 128], f32, tag="wst")
        nc.sync.dma_start(out=w_stage, in_=wqkv.rearrange("w c e d -> e w c d"))
        for i, w_sb in enumerate((wq_sb, wk_sb, wv_sb)):
            nc.vector.tensor_copy(out=w_sb, in_=w_stage[:, i])
        wo_f32 = wst_p.tile([64, 2, EMBED], f32, tag="wof")
        nc.sync.dma_start(out=wo_f32, in_=wo)
        nc.vector.tensor_copy(out=wo_sb, in_=wo_f32)
    nc.sync.dma_start(out=id_sb, in_=ident)
    nc.sync.dma_start(out=bq_sb, in_=bqk[0:1, :].rearrange("a p -> p a"))
    nc.sync.dma_start(out=bk_sb, in_=bqk[1:2, :].rearrange("a p -> p a"))
    bv_bcast = bass.AP(tensor=bv.tensor, offset=bv.offset, ap=[[0, 128], [1, 128]])
    nc.sync.dma_start(out=bv_sb, in_=bv_bcast)
    ones_sb = persist.tile([128, 1], f32, tag="ones")
    nc.vector.memset(ones_sb, 1.0)
    vP_ones = vP.rearrange("p c (h e) -> p c h e", h=2)[:, :, :, HD : HD + 1]
    ones_b = bass.AP(
        tensor=ones_sb.tensor, offset=ones_sb.offset,
        ap=[ones_sb.ap[0], [0, TC], [0, 2], [0, 1]],
    )
    nc.vector.tensor_copy(out=vP_ones, in_=ones_b)
    def ht_block(b):
        # hidden^T for s-block b via DMA xbar transpose (2-byte dtype)
        for c in range(EC):
            nc.sync.dma_start(
                out=hTa[:, c, ts(b, SUP)],
                in_=hidB[ts(b, SUP), ts(c, 128)],
                transpose=True,
            )

    pT_p = st_.enter_context(tc.tile_pool(name="pT", bufs=2))
    dtmp_p = st_.enter_context(tc.tile_pool(name="dtmp", bufs=2))
    rD_p = st_.enter_context(tc.tile_pool(name="rD", bufs=2))
    an_p = st_.enter_context(tc.tile_pool(name="anorm", bufs=3))
    stg_p = st_.enter_context(tc.tile_pool(name="stage", bufs=2))
    sc_ps_p = st_.enter_context(tc.tile_pool(name="ps_sc", bufs=2, space="PSUM"))
    at_ps_p = st_.enter_context(tc.tile_pool(name="ps_at", bufs=1, space="PSUM"))
    aux_ps_p = st_.enter_context(tc.tile_pool(name="ps_aux", bufs=2, space="PSUM"))

    rd_of = {}
    at_of = {}
    d_of = {}

    def q_proj(sup):
        q_ps = aux_ps_p.tile([128, SUP], f32, tag="aux", name=f"q_ps{sup}")
        for c in range(EC):
            nc.tensor.matmul(
                q_ps, wq_sb[:, c, :], hTa[:, c, ts(sup, SUP)],
                start=(c == 0), stop=(c == EC - 1),
            )
        nc.vector.tensor_scalar(
            out=qT[:, ts(sup, SUP)], in0=q_ps,
            scalar1=bq_sb, scalar2=None, op0=AOT.add,
        )

    def kv_block(b):
        k_ps = aux_ps_p.tile([128, SUP], f32, tag="aux", name=f"k_ps{b}")
        for c in range(EC):
            nc.tensor.matmul(
                k_ps, wk_sb[:, c, :], hTa[:, c, ts(b, SUP)],
                start=(c == 0), stop=(c == EC - 1),
            )
        nc.vector.tensor_scalar(
            out=kT[:, ts(b, SUP)], in0=k_ps,
            scalar1=bk_sb, scalar2=None, op0=AOT.add,
        )
        v_ps = aux_ps_p.tile([128, SUP], f32, tag="aux", name=f"v_ps{b}")
        for j in range(JS):
            for c in range(EC):
                nc.tensor.matmul(
                    v_ps[:, ts(j, 128)],
                    hTa[:, c, ts(JS * b + j, 128)],
                    wv_sb[:, c, :],
                    start=(c == 0), stop=(c == EC - 1),
                    skip_group_check=True,
                )
        for j in range(JS):
            t_idx = JS * b + j
            dst = vP[:, t_idx, :].rearrange("p (h e) -> p h e", h=2)[:, :, 0:HD]
            nc.vector.tensor_add(
                out=dst,
                in0=v_ps[:, ts(j, 128)].rearrange("p (h d) -> p h d", h=2),
                in1=bv_sb.rearrange("p (h d) -> p h d", h=2),
            )

    def b_chunk(sup, c):
        sc_ps = sc_ps_p.tile([128, 2 * SUP], f32, tag="sc", name=f"sc{sup}_{c}")
        for h in range(HPC):
            nc.tensor.matmul(
                sc_ps[:, ts(h, SUP)],
                kT[ts(h, HD), ts(c, 128)],
                qT[ts(h, HD), ts(sup, SUP)],
                start=True, stop=True,
                tile_position=(h * HD, 0),
            )
        pT = pT_p.tile([128, 2 * SUP], bf16, tag="pT", name=f"pT{sup}_{c}")
        nc.scalar.activation(out=pT, in_=sc_ps, func=Exp)
        for h in range(HPC):
            nc.tensor.matmul(
                at_of[sup][h],
                vP[:, c, ts(h, HD + 1)],
                pT[:, ts(h, SUP)],
                start=(c == 0), stop=(c == TC - 1),
            )

    def drain(sup):
        dts = [
            dtmp_p.tile([1, SUP], f32, tag=f"d{h}", name=f"d{sup}_{h}")
            for h in range(HPC)
        ]
        d_of[sup] = dts
        for h in range(HPC):
            nc.vector.tensor_copy(out=xT[h][:, ts(sup, SUP)], in_=at_of[sup][h])
            nc.vector.tensor_copy(out=dts[h], in_=at_of[sup][h][HD : HD + 1, :])

    def c_head(sup):
        # denominators -> s-partitioned reciprocals (reads xT row 64)
        dT_ps = sc_ps_p.tile([128, HPC * JS], f32, tag="sc", name=f"dT{sup}")
        for h in range(HPC):
            for j in range(JS):
                nc.tensor.transpose(
                    dT_ps[:, h * JS + j : h * JS + j + 1],
                    d_of[sup][h][:, ts(j, 128)],
                    id_sb[0:1, 0:1],
                )
        rD = rD_p.tile([128, HPC, JS], f32, tag="rD", name=f"rD{sup}")
        nc.vector.reciprocal(out=rD.rearrange("p h j -> p (h j)"), in_=dT_ps)
        rd_of[sup] = rD

    def c_unit(sup, j, eh, stage):
        st_i = JS * sup + j
        rD = rd_of[sup]
        o_ps = [
            aux_ps_p.tile([128, SUP], f32, tag="aux", name=f"o{st_i}_{eh}_{h}")
            for h in range(HPC)
        ]
        for h in range(HPC):
            nc.tensor.matmul(
                o_ps[h],
                xT[h][0:HD, ts(st_i, 128)],
                wo_sb[:, h, ts(eh, SUP)],
                start=True, stop=True,
            )
        t0 = an_p.tile([128, SUP], f32, tag="t0", name=f"t0_{st_i}_{eh}")
        t1 = an_p.tile([128, SUP], f32, tag="t1", name=f"t1_{st_i}_{eh}")
        nc.vector.tensor_scalar_mul(out=t0, in0=o_ps[0], scalar1=rD[:, 0, j : j + 1])
        nc.vector.tensor_scalar_mul(out=t1, in0=o_ps[1], scalar1=rD[:, 1, j : j + 1])
        nc.vector.tensor_add(out=stage[:, ts(eh, SUP)], in0=t0, in1=t1)
        if eh == EMBED // SUP - 1:
            nc.sync.dma_start(out=outp[ts(st_i, 128), :], in_=stage)

    def c_tail(sup, slot):
        # slot 0: head (D/recip); slots 1..8: the 8 (j, eh) units
        if slot == 0:
            c_head(sup)
        else:
            u = slot - 1
            j, eh = divmod(u, EMBED // SUP)
            if eh == 0:
                stage_tiles[sup % 2][j] = stg_p.tile(
                    [128, EMBED], f32, tag="stage", name=f"stage{sup}_{j}"
                )
            c_unit(sup, j, eh, stage_tiles[sup % 2][j])

    stage_tiles = [[None] * JS, [None] * JS]

    # ---- phase A interleaved with super 0 (lag-one chunk groups) ---------
    at_of[0] = [
        at_ps_p.tile([HD + 1, SUP], f32, tag=f"at{h}", name=f"at0_{h}")
        for h in range(HPC)
    ]
    ht_block(0)
    kv_block(0)
    q_proj(0)
    for b in range(1, NSUP):
        ht_block(b)
        for c in range(JS * (b - 1), JS * b):
            b_chunk(0, c)
        kv_block(b)
    for c in range(JS * (NSUP - 1), SEQ // 128):
        b_chunk(0, c)
    q_proj(1)

    # ---- supers 1..7 with trailing C(sup-1), q_proj(sup+1) in-stream -----
    for sup in range(1, NSUP):
        drain(sup - 1)
        at_of[sup] = [
            at_ps_p.tile([HD + 1, SUP], f32, tag=f"at{h}", name=f"at{sup}_{h}")
            for h in range(HPC)
        ]
        slot = 0
        for c in range(TC):
            b_chunk(sup, c)
            if c % 3 == 2 and slot < 9:
                c_tail(sup - 1, slot)
                slot += 1
            if c == 29 and sup + 1 < NSUP:
                q_proj(sup + 1)
        while slot < 9:
            c_tail(sup - 1, slot)
            slot += 1
    drain(NSUP - 1)
    for slot in range(9):
        c_tail(NSUP - 1, slot)

    st_.close()


def _shards(inputs):
    """Host-side prep: per-core input dicts (head-parallel, Wo row-shard)."""
    hs = np.ascontiguousarray(np.asarray(inputs["hidden_state"], np.float32))
    Wq = np.asarray(inputs["Wq"], np.float32) * 0.125  # fold 1/sqrt(64); exact
    bq = np.asarray(inputs["bq"], np.float32) * 0.125
    Wk = np.asarray(inputs["Wk"], np.float32)
    bk = np.asarray(inputs["bk"], np.float32)
    Wv = np.asarray(inputs["Wv"], np.float32)
    bv = np.asarray(inputs["bv"], np.float32)
    Wo = np.asarray(inputs["Wo"], np.float32)
    ident = np.eye(128, dtype=np.float32)
    hs_bf16 = None
    if PV_DT == "bf16":
        import ml_dtypes

        hs_bf16 = np.ascontiguousarray(hs.astype(ml_dtypes.bfloat16))

    in_maps = []
    for c in range(NCORES):
        h0 = HPC * c
        # [H,E,Dh] head-pair -> [E, 2*Dh] -> [EC, 128, 128]
        def _w(W):
            w = np.transpose(W[h0 : h0 + HPC], (1, 0, 2)).reshape(EMBED, 128)
            return np.ascontiguousarray(w.reshape(EC, 128, 128))

        w_qkv = np.stack([_w(Wq), _w(Wk), _w(Wv)])
        b_qk = np.stack(
            [bq[h0 : h0 + HPC].reshape(128), bk[h0 : h0 + HPC].reshape(128)]
        )
        b_v = np.ascontiguousarray(bv[h0 : h0 + HPC].reshape(128))
        # Wo rows for this core's heads: [128, E] -> [64, 2, E] (head-major)
        w_o = np.ascontiguousarray(
            Wo[128 * c : 128 * (c + 1)].reshape(2, 64, EMBED).transpose(1, 0, 2)
        )
        im = (
            {"hidden_bf16": hs_bf16} if PV_DT == "bf16" else {"hidden": hs}
        )
        in_maps.append(
            {
                **im,
                "w_qkv": np.ascontiguousarray(w_qkv),
                "b_qk": np.ascontiguousarray(b_qk),
                "b_v": b_v,
                "w_o": w_o,
                "ident": ident,
            }
        )
    return in_maps


def kernel(**inputs):
    global LAST
    from concourse import bass_utils

    trace = bool(int(os.environ.get("K_TRACE", "0")))
    if trace:
        _install_ntff_shim()

    key = (MM_DT, PV_DT)
    if key not in _CACHE:
        _CACHE[key] = _build()
    nc = _CACHE[key]

    in_maps = _shards(inputs)
    res = bass_utils.run_bass_kernel_spmd(
        nc, in_maps, core_ids=list(range(NCORES)), trace=trace
    )
    LAST = res

    out = np.zeros((SEQ, EMBED), np.float64)
    for c in range(NCORES):
        out += res.results[c]["out_p"].astype(np.float64)
    out += np.asarray(inputs["bo"], np.float32).astype(np.float64)
    return out.astype(np.float32)


def _install_ntff_shim():
    """antenv.axon_hooks is absent from this image; recreate it so
    run_bass_kernel_spmd(trace=True) can reach the NTFF profiling hook."""
    import types

    if "antenv.axon_hooks" in sys.modules:
        return
    try:
        if "/root/.axon_site" not in sys.path:
            sys.path.insert(0, "/root/.axon_site")
        from trn_agent_boot.trn_boot import _ntff_profile_via_ctypes

        hook = _ntff_profile_via_ctypes("/opt/axon/libaxon_pjrt.so")
    except Exception:
        hook = None
    mod = types.ModuleType("antenv.axon_hooks")
    mod._hook = hook
    mod.get_axon_ntff_profile_hook = lambda: mod._hook
    mod.set_axon_ntff_profile_hook = lambda h: setattr(mod, "_hook", h)
    sys.modules["antenv.axon_hooks"] = mod


# revision 17
# speedup vs baseline: 1.6665x; 1.0099x over previous
"""Multi-head attention (SEQ=4096, EMBED=1024, 16 heads, Dh=64) on 8 TRN2
NeuronCores, head-parallel: 2 heads per core, Wo row-sharded so each core
emits a partial output [SEQ, EMBED]; the host sums the 8 partials (+bo).

Per-core pipeline (one TileContext):
  A) hidden^T e-chunks via PE transposes (fp32 has no DMA transpose);
     projections Q^T,K^T [128(=2 heads x 64 d), S] and V' [t, 2*(64+ones)]
     with the bias fused into the PSUM->SBUF copy. The 1/sqrt(Dh) scale is
     folded into Wq/bq on the host (exact: *0.125).
  B) per s-super(512): scores^T [t, s] as two row-tiled K=64 matmuls (the
     two heads concurrently fill the 128-row array, tile_position (0,0) /
     (64,0)); one Exp per t-chunk over the [128, 2*512] PSUM pair (logits
     are bounded ~|3|, so no max subtraction needed); attn'^T [65, s]
     accumulated in PSUM where the ones column of V' makes row 64 the
     softmax denominator.
  C) denominators -> tiny PE transposes -> DVE reciprocal (s-partitioned);
     Wo per head into separate PSUM banks; DVE scales each head's partial
     by its 1/D and sums; DMA out.
"""

import os
import sys

sys.path.insert(0, "/opt/trn_rl_repo")

import numpy as np

SEQ = 4096
EMBED = 1024
HEADS = 16
HD = 64
NCORES = 8
HPC = HEADS // NCORES  # 2 heads per core
EC = EMBED // 128  # 8 e-chunks
SUP = 512  # s-super size
NSUP = SEQ // SUP  # 8
TC = SEQ // 128  # 32 t-chunks
JS = SUP // 128  # 4 s-tiles per super

# dtype knobs: f32r = fp32 storage, single-pass reduced-precision matmul
MM_DT = os.environ.get("K_MM_DT", "f32r")  # f32r | f32
PV_DT = os.environ.get("K_PV_DT", "f32")  # f32 | bf16 (P^T/V'/x^T/Wo storage)

LAST = None  # BassKernelResults of the most recent run (read by test.py)
_CACHE = {}


def _mm(ap):
    """View an fp32 AP as float32r for single-pass matmuls, if enabled."""
    from concourse import mybir

    if MM_DT == "f32r" and ap.dtype == mybir.dt.float32:
        return ap.bitcast(mybir.dt.float32r)
    return ap


def _build():
    import concourse.bacc as bacc
    import concourse.tile as tile
    from concourse import mybir

    f32 = mybir.dt.float32

    nc = bacc.Bacc("TRN2", debug=False, enable_asserts=False, num_devices=NCORES)

    wqkv = nc.dram_tensor("w_qkv", [3, EC, 128, 128], f32, kind="ExternalInput").ap()
    bqk = nc.dram_tensor("b_qk", [2, 128], f32, kind="ExternalInput").ap()
    bv = nc.dram_tensor("b_v", [128], f32, kind="ExternalInput").ap()
    wo = nc.dram_tensor("w_o", [64, 2, EMBED], f32, kind="ExternalInput").ap()
    ident = nc.dram_tensor("ident", [128, 128], f32, kind="ExternalInput").ap()
    outp = nc.dram_tensor("out_p", [SEQ, EMBED], f32, kind="ExternalOutput").ap()

    if PV_DT == "bf16":
        hidB = nc.dram_tensor(
            "hidden_bf16", [SEQ, EMBED], mybir.dt.bfloat16, kind="ExternalInput"
        ).ap()
        with tile.TileContext(nc) as tc:
            _emit_bf16(tc, mybir, hidB, wqkv, bqk, bv, wo, ident, outp)
    else:
        hid = nc.dram_tensor("hidden", [SEQ, EMBED], f32, kind="ExternalInput").ap()
        with tile.TileContext(nc) as tc:
            _emit(tc, mybir, hid, wqkv, bqk, bv, wo, ident, outp)

    nc.compile()
    return nc


def _emit(tc, mybir, hid, wqkv, bqk, bv, wo, ident, outp):
    import concourse.bass as bass

    nc = tc.nc
    ts = bass.ts
    f32 = mybir.dt.float32
    f32r = mybir.dt.float32r
    mmdt = f32r if MM_DT == "f32r" else f32
    # attention-side storage: bf16 if requested, else the matmul fp32 flavor
    pv = mybir.dt.bfloat16 if PV_DT == "bf16" else mmdt
    projdt = mybir.dt.bfloat16 if PV_DT == "bf16" else mmdt
    Exp = mybir.ActivationFunctionType.Exp
    AOT = mybir.AluOpType

    # ---- persistent tiles -------------------------------------------------
    import contextlib

    _stack = contextlib.ExitStack()
    persist = _stack.enter_context(tc.tile_pool(name="persist", bufs=1))
    qT = persist.tile([128, SEQ], pv, tag="qT")  # [(h,d), s]
    kT = persist.tile([128, SEQ], pv, tag="kT")
    vP = persist.tile([128, TC, 2 * (HD + 1)], pv, tag="vP")  # V' per t-chunk
    xT = [persist.tile([HD + 1, SEQ], pv, tag=f"xT{h}", name=f"xT{h}") for h in range(HPC)]
    wq_sb = persist.tile([128, EC, 128], projdt, tag="wq")
    wk_sb = persist.tile([128, EC, 128], projdt, tag="wk")
    wv_sb = persist.tile([128, EC, 128], projdt, tag="wv")
    wo_sb = persist.tile([64, 2, EMBED], pv, tag="wo")
    id_sb = persist.tile([128, 128], f32, tag="ident")
    bq_sb = persist.tile([128, 1], f32, tag="bq")
    bk_sb = persist.tile([128, 1], f32, tag="bk")
    bv_sb = persist.tile([128, 128], f32, tag="bv")

    w_stage = persist.tile([128, 3, EC, 128], f32, tag="w_stage")
    nc.sync.dma_start(out=w_stage, in_=wqkv.rearrange("w c e d -> e w c d"))
    for i, w_sb in enumerate((wq_sb, wk_sb, wv_sb)):
        if w_sb.dtype == f32:
            nc.sync.dma_start(out=w_sb, in_=wqkv[i].rearrange("c e d -> e c d"))
        else:
            nc.vector.tensor_copy(out=w_sb, in_=w_stage[:, i])
    if pv == f32:
        nc.sync.dma_start(out=wo_sb, in_=wo)
    else:
        wo_f32 = persist.tile([64, 2, EMBED], f32, tag="wo_f32")
        nc.sync.dma_start(out=wo_f32, in_=wo)
        nc.vector.tensor_copy(out=wo_sb, in_=wo_f32)
    nc.sync.dma_start(out=id_sb, in_=ident)
    nc.sync.dma_start(out=bq_sb, in_=bqk[0:1, :].rearrange("a p -> p a"))
    nc.sync.dma_start(out=bk_sb, in_=bqk[1:2, :].rearrange("a p -> p a"))
    bv_bcast = bass.AP(tensor=bv.tensor, offset=bv.offset, ap=[[0, 128], [1, 128]])
    nc.sync.dma_start(out=bv_sb, in_=bv_bcast)
    # ones columns of V' (free positions h*65+64); V overwrites cols 0..63
    ones_sb = persist.tile([128, 1], f32, tag="ones")
    nc.vector.memset(ones_sb, 1.0)
    vP_ones = vP.rearrange("p c (h e) -> p c h e", h=2)[:, :, :, HD : HD + 1]
    ones_b = bass.AP(
        tensor=ones_sb.tensor,
        offset=ones_sb.offset,
        ap=[ones_sb.ap[0], [0, TC], [0, 2], [0, 1]],
    )
    nc.vector.tensor_copy(out=vP_ones, in_=ones_b)

    # ---- phase A: hidden^T + projections ---------------------------------
    with (
        tc.tile_pool(name="hnat", bufs=4) as hnat_p,
        tc.tile_pool(name="hT", bufs=2) as hT_p,
        tc.tile_pool(name="ps_tp", bufs=2, space="PSUM") as tp_ps_p,
        tc.tile_pool(name="ps_proj", bufs=2, space="PSUM") as proj_ps_p,
    ):
        for sb in range(NSUP):  # s-blocks of 512
            hn = [hnat_p.tile([128, EMBED], f32, tag="hn", name=f"hn{sb}_{_j}") for _j in range(JS)]
            for j in range(JS):
                nc.sync.dma_start(out=hn[j], in_=hid[ts(JS * sb + j, 128), :])
            hT = hT_p.tile([128, EC, SUP], projdt, tag="hT")  # [e, chunk, s]
            for c in range(EC):
                tp = tp_ps_p.tile([128, SUP], f32, tag="tp")
                for j in range(JS):
                    nc.tensor.transpose(
                        tp[:, ts(j, 128)], hn[j][:, ts(c, 128)], id_sb
                    )
                nc.vector.tensor_copy(out=hT[:, c, :], in_=tp)
            q_ps = proj_ps_p.tile([128, SUP], f32, tag="q_ps")
            k_ps = proj_ps_p.tile([128, SUP], f32, tag="k_ps")
            v_ps = proj_ps_p.tile([128, SUP], f32, tag="v_ps")
            for c in range(EC):
                nc.tensor.matmul(
                    q_ps, _mm(wq_sb[:, c, :]), _mm(hT[:, c, :]),
                    start=(c == 0), stop=(c == EC - 1),
                )
            for c in range(EC):
                nc.tensor.matmul(
                    k_ps, _mm(wk_sb[:, c, :]), _mm(hT[:, c, :]),
                    start=(c == 0), stop=(c == EC - 1),
                )
            for j in range(JS):  # V natural [t, d], 4 t-tiles per block
                for c in range(EC):
                    nc.tensor.matmul(
                        v_ps[:, ts(j, 128)],
                        _mm(hT[:, c, ts(j, 128)]),
                        _mm(wv_sb[:, c, :]),
                        start=(c == 0), stop=(c == EC - 1),
                        skip_group_check=True,
                    )
            # PSUM -> SBUF with fused bias (and optional bf16 downcast)
            nc.vector.tensor_scalar(
                out=qT[:, ts(sb, SUP)], in0=q_ps,
                scalar1=bq_sb, scalar2=None, op0=AOT.add,
            )
            nc.vector.tensor_scalar(
                out=kT[:, ts(sb, SUP)], in0=k_ps,
                scalar1=bk_sb, scalar2=None, op0=AOT.add,
            )
            for j in range(JS):
                t_idx = JS * sb + j
                dst = vP[:, t_idx, :].rearrange("p (h e) -> p h e", h=2)[:, :, 0:HD]
                nc.vector.tensor_add(
                    out=dst,
                    in0=v_ps[:, ts(j, 128)].rearrange("p (h d) -> p h d", h=2),
                    in1=bv_sb.rearrange("p (h d) -> p h d", h=2),
                )

    # ---- phases B+C: attention + output, per s-super ---------------------
    with (
        tc.tile_pool(name="pT", bufs=2) as pT_p,
        tc.tile_pool(name="dtmp", bufs=2) as dtmp_p,
        tc.tile_pool(name="rD", bufs=2) as rD_p,
        tc.tile_pool(name="anorm", bufs=3) as an_p,
        tc.tile_pool(name="stage", bufs=2) as st_p,
        tc.tile_pool(name="ps_sc", bufs=2, space="PSUM") as sc_ps_p,
        tc.tile_pool(name="ps_at", bufs=1, space="PSUM") as at_ps_p,
        tc.tile_pool(name="ps_wo", bufs=1, space="PSUM") as wo_ps_p,
    ):
        for sup in range(NSUP):
            ssl = ts(sup, SUP)
            at_ps = [
                at_ps_p.tile([HD + 1, SUP], f32, tag=f"at{h}", name=f"at{sup}_{h}") for h in range(HPC)
            ]
            for c in range(TC):
                sc_ps = sc_ps_p.tile([128, 2 * SUP], f32, tag="sc")
                for h in range(HPC):
                    nc.tensor.matmul(
                        sc_ps[:, ts(h, SUP)],
                        _mm(kT[ts(h, HD), ts(c, 128)]),
                        _mm(qT[ts(h, HD), ssl]),
                        start=True, stop=True,
                        tile_position=(h * HD, 0),
                    )
                pT = pT_p.tile([128, 2 * SUP], pv, tag="pT")
                nc.scalar.activation(out=pT, in_=sc_ps, func=Exp)
                for h in range(HPC):
                    nc.tensor.matmul(
                        at_ps[h],
                        _mm(vP[:, c, ts(h, HD + 1)]),
                        _mm(pT[:, ts(h, SUP)]),
                        start=(c == 0), stop=(c == TC - 1),
                    )
            # attn'^T -> SBUF; row 64 is the softmax denominator
            d_tmp = [
                dtmp_p.tile([1, SUP], f32, tag=f"d{h}", name=f"d{sup}_{h}")
                for h in range(HPC)
            ]
            for h in range(HPC):
                nc.vector.tensor_copy(out=xT[h][:, ssl], in_=at_ps[h])
                nc.vector.tensor_copy(
                    out=d_tmp[h], in_=at_ps[h][HD : HD + 1, :]
                )
            # denominators -> s-partitioned layout via tiny PE transposes
            dT_ps = sc_ps_p.tile([128, HPC * JS], f32, tag="sc")
            for h in range(HPC):
                for j in range(JS):
                    nc.tensor.transpose(
                        dT_ps[:, h * JS + j : h * JS + j + 1],
                        d_tmp[h][:, ts(j, 128)],
                        id_sb[0:1, 0:1],
                    )
            rD = rD_p.tile([128, HPC, JS], f32, tag="rD")
            nc.vector.reciprocal(
                out=rD.rearrange("p h j -> p (h j)"), in_=dT_ps
            )
            # Wo per head, then scale by 1/D_h and add heads
            for j in range(JS):
                st_i = JS * sup + j
                stage = st_p.tile([128, EMBED], f32, tag="stage")
                for eh in range(EMBED // SUP):
                    o_ps = [
                        wo_ps_p.tile([128, SUP], f32, tag=f"o{h}", name=f"o{st_i}_{eh}_{h}")
                        for h in range(HPC)
                    ]
                    for h in range(HPC):
                        nc.tensor.matmul(
                            o_ps[h],
                            _mm(xT[h][0:HD, ts(st_i, 128)]),
                            _mm(wo_sb[:, h, ts(eh, SUP)]),
                            start=True, stop=True,
                        )
                    t0 = an_p.tile([128, SUP], f32, tag="t0")
                    t1 = an_p.tile([128, SUP], f32, tag="t1")
                    nc.vector.tensor_scalar_mul(
                        out=t0, in0=o_ps[0], scalar1=rD[:, 0, j : j + 1]
                    )
                    nc.vector.tensor_scalar_mul(
                        out=t1, in0=o_ps[1], scalar1=rD[:, 1, j : j + 1]
                    )
                    nc.vector.tensor_add(
                        out=stage[:, ts(eh, SUP)], in0=t0, in1=t1
                    )
                nc.sync.dma_start(out=outp[ts(st_i, 128), :], in_=stage)

    _stack.close()


def _emit_bf16(tc, mybir, hidB, wqkv, bqk, bv, wo, ident, outp):
    """bf16 path: DMA-xbar-transposed hidden, A/B interleave, trailing C."""
    import concourse.bass as bass

    nc = tc.nc
    ts = bass.ts
    f32 = mybir.dt.float32
    bf16 = mybir.dt.bfloat16
    Exp = mybir.ActivationFunctionType.Exp
    AOT = mybir.AluOpType

    import contextlib

    st_ = contextlib.ExitStack()
    persist = st_.enter_context(tc.tile_pool(name="persist", bufs=1))
    qT = persist.tile([128, SEQ], bf16, tag="qT")
    kT = persist.tile([128, SEQ], bf16, tag="kT")
    vP = persist.tile([128, TC, 2 * (HD + 1)], bf16, tag="vP")
    xT = [persist.tile([HD + 1, SEQ], bf16, tag=f"xT{h}", name=f"xT{h}") for h in range(HPC)]
    hTa = persist.tile([128, EC, SEQ], bf16, tag="hTa")  # hidden^T, all chunks
    wq_sb = persist.tile([128, EC, 128], bf16, tag="wq")
    wk_sb = persist.tile([128, EC, 128], bf16, tag="wk")
    wv_sb = persist.tile([128, EC, 128], bf16, tag="wv")
    wo_sb = persist.tile([64, 2, EMBED], bf16, tag="wo")
    id_sb = persist.tile([128, 128], f32, tag="ident")
    bq_sb = persist.tile([128, 1], f32, tag="bq")
    bk_sb = persist.tile([128, 1], f32, tag="bk")
    bv_sb = persist.tile([128, 1], f32, tag="bv")
    idb_sb = persist.tile([128, 128], bf16, tag="idb")

    with tc.tile_pool(name="wstage", bufs=1) as wst_p:
        w_stage = wst_p.tile([128, 3, EC, 128], f32, tag="wst")
        nc.sync.dma_start(out=w_stage, in_=wqkv.rearrange("w c e d -> e w c d"))
        for i, w_sb in enumerate((wq_sb, wk_sb, wv_sb)):
            nc.vector.tensor_copy(out=w_sb, in_=w_stage[:, i])
        wo_f32 = wst_p.tile([64, 2, EMBED], f32, tag="wof")
        nc.sync.dma_start(out=wo_f32, in_=wo)
        nc.vector.tensor_copy(out=wo_sb, in_=wo_f32)
    nc.sync.dma_start(out=id_sb, in_=ident)
    nc.vector.tensor_copy(out=idb_sb, in_=id_sb)
    nc.sync.dma_start(out=bq_sb, in_=bqk[0:1, :].rearrange("a p -> p a"))
    nc.sync.dma_start(out=bk_sb, in_=bqk[1:2, :].rearrange("a p -> p a"))
    bv_col = bass.AP(tensor=bv.tensor, offset=bv.offset, ap=[[1, 128], [1, 1]])
    nc.sync.dma_start(out=bv_sb, in_=bv_col)
    ones_sb = persist.tile([128, 1], f32, tag="ones")
    nc.vector.memset(ones_sb, 1.0)
    vP_ones = vP.rearrange("p c (h e) -> p c h e", h=2)[:, :, :, HD : HD + 1]
    ones_b = bass.AP(
        tensor=ones_sb.tensor, offset=ones_sb.offset,
        ap=[ones_sb.ap[0], [0, TC], [0, 2], [0, 1]],
    )
    nc.vector.tensor_copy(out=vP_ones, in_=ones_b)
    def ht_block(b):
        # hidden^T for s-block b via DMA xbar transpose (2-byte dtype)
        for c in range(EC):
            nc.sync.dma_start(
                out=hTa[:, c, ts(b, SUP)],
                in_=hidB[ts(b, SUP), ts(c, 128)],
                transpose=True,
            )

    pT_p = st_.enter_context(tc.tile_pool(name="pT", bufs=2))
    vT_p = st_.enter_context(tc.tile_pool(name="vT", bufs=2))
    dtmp_p = st_.enter_context(tc.tile_pool(name="dtmp", bufs=2))
    rD_p = st_.enter_context(tc.tile_pool(name="rD", bufs=2))
    an_p = st_.enter_context(tc.tile_pool(name="anorm", bufs=3))
    stg_p = st_.enter_context(tc.tile_pool(name="stage", bufs=2))
    sc_ps_p = st_.enter_context(tc.tile_pool(name="ps_sc", bufs=2, space="PSUM"))
    at_ps_p = st_.enter_context(tc.tile_pool(name="ps_at", bufs=1, space="PSUM"))
    aux_ps_p = st_.enter_context(tc.tile_pool(name="ps_aux", bufs=2, space="PSUM"))

    rd_of = {}
    at_of = {}
    d_of = {}

    def q_proj(sup):
        q_ps = aux_ps_p.tile([128, SUP], f32, tag="aux", name=f"q_ps{sup}")
        for c in range(EC):
            nc.tensor.matmul(
                q_ps, wq_sb[:, c, :], hTa[:, c, ts(sup, SUP)],
                start=(c == 0), stop=(c == EC - 1),
            )
        nc.vector.tensor_scalar(
            out=qT[:, ts(sup, SUP)], in0=q_ps,
            scalar1=bq_sb, scalar2=None, op0=AOT.add,
        )

    def kv_block(b):
        k_ps = aux_ps_p.tile([128, SUP], f32, tag="aux", name=f"k_ps{b}")
        for c in range(EC):
            nc.tensor.matmul(
                k_ps, wk_sb[:, c, :], hTa[:, c, ts(b, SUP)],
                start=(c == 0), stop=(c == EC - 1),
            )
        nc.vector.tensor_scalar(
            out=kT[:, ts(b, SUP)], in0=k_ps,
            scalar1=bk_sb, scalar2=None, op0=AOT.add,
        )
        vT_ps = aux_ps_p.tile([128, SUP], f32, tag="aux", name=f"vT_ps{b}")
        for c in range(EC):
            nc.tensor.matmul(
                vT_ps, wv_sb[:, c, :], hTa[:, c, ts(b, SUP)],
                start=(c == 0), stop=(c == EC - 1),
            )
        vT_sb = vT_p.tile([128, SUP], bf16, tag="vT", name=f"vT{b}")
        nc.vector.tensor_scalar(
            out=vT_sb, in0=vT_ps, scalar1=bv_sb, scalar2=None, op0=AOT.add
        )
        tp_ps = aux_ps_p.tile([128, JS, 128], bf16, tag="aux", name=f"tp_ps{b}")
        for j in range(JS):
            nc.tensor.transpose(tp_ps[:, j, :], vT_sb[:, ts(j, 128)], idb_sb)
        for j in range(JS):
            t_idx = JS * b + j
            dst = vP[:, t_idx, :].rearrange("p (h e) -> p h e", h=2)[:, :, 0:HD]
            nc.vector.tensor_copy(
                out=dst,
                in_=tp_ps[:, j, :].rearrange("p (h d) -> p h d", h=2),
            )

    pT_of = {}

    def sc_exp(sup, c):
        sc_ps = sc_ps_p.tile([128, 2 * SUP], f32, tag="sc", name=f"sc{sup}_{c}")
        for h in range(HPC):
            nc.tensor.matmul(
                sc_ps[:, ts(h, SUP)],
                kT[ts(h, HD), ts(c, 128)],
                qT[ts(h, HD), ts(sup, SUP)],
                start=True, stop=True,
                tile_position=(h * HD, 0),
            )
        pT = pT_p.tile([128, 2 * SUP], bf16, tag="pT", name=f"pT{sup}_{c}")
        nc.scalar.activation(out=pT, in_=sc_ps, func=Exp)
        pT_of[(sup, c)] = pT

    def at_mms(sup, c):
        pT = pT_of.pop((sup, c))
        for h in range(HPC):
            nc.tensor.matmul(
                at_of[sup][h],
                vP[:, c, ts(h, HD + 1)],
                pT[:, ts(h, SUP)],
                start=(c == 0), stop=(c == TC - 1),
            )

    def drain(sup):
        dts = [
            dtmp_p.tile([1, SUP], f32, tag=f"d{h}", name=f"d{sup}_{h}")
            for h in range(HPC)
        ]
        d_of[sup] = dts
        for h in range(HPC):
            nc.vector.tensor_copy(out=xT[h][:, ts(sup, SUP)], in_=at_of[sup][h])
            nc.vector.tensor_copy(out=dts[h], in_=at_of[sup][h][HD : HD + 1, :])

    def c_head(sup):
        # denominators -> s-partitioned reciprocals (reads xT row 64)
        dT_ps = sc_ps_p.tile([128, HPC * JS], f32, tag="sc", name=f"dT{sup}")
        for h in range(HPC):
            for j in range(JS):
                nc.tensor.transpose(
                    dT_ps[:, h * JS + j : h * JS + j + 1],
                    d_of[sup][h][:, ts(j, 128)],
                    id_sb[0:1, 0:1],
                )
        rD = rD_p.tile([128, HPC, JS], f32, tag="rD", name=f"rD{sup}")
        nc.vector.reciprocal(out=rD.rearrange("p h j -> p (h j)"), in_=dT_ps)
        rd_of[sup] = rD

    def c_unit(sup, j, eh, stage):
        st_i = JS * sup + j
        rD = rd_of[sup]
        o_ps = [
            aux_ps_p.tile([128, SUP], f32, tag="aux", name=f"o{st_i}_{eh}_{h}")
            for h in range(HPC)
        ]
        for h in range(HPC):
            nc.tensor.matmul(
                o_ps[h],
                xT[h][0:HD, ts(st_i, 128)],
                wo_sb[:, h, ts(eh, SUP)],
                start=True, stop=True,
            )
        t0 = an_p.tile([128, SUP], f32, tag="t0", name=f"t0_{st_i}_{eh}")
        t1 = an_p.tile([128, SUP], f32, tag="t1", name=f"t1_{st_i}_{eh}")
        nc.vector.tensor_scalar_mul(out=t0, in0=o_ps[0], scalar1=rD[:, 0, j : j + 1])
        nc.vector.tensor_scalar_mul(out=t1, in0=o_ps[1], scalar1=rD[:, 1, j : j + 1])
        nc.vector.tensor_add(out=stage[:, ts(eh, SUP)], in0=t0, in1=t1)
        if eh == EMBED // SUP - 1:
            nc.sync.dma_start(out=outp[ts(st_i, 128), :], in_=stage)

    def c_tail(sup, slot):
        # slot 0: head (D/recip); slots 1..8: the 8 (j, eh) units
        if slot == 0:
            c_head(sup)
        else:
            u = slot - 1
            j, eh = divmod(u, EMBED // SUP)
            if eh == 0:
                stage_tiles[sup % 2][j] = stg_p.tile(
                    [128, EMBED], f32, tag="stage", name=f"stage{sup}_{j}"
                )
            c_unit(sup, j, eh, stage_tiles[sup % 2][j])

    stage_tiles = [[None] * JS, [None] * JS]

    # ---- phase A interleaved with super 0 (lag-one chunk groups) ---------
    at_of[0] = [
        at_ps_p.tile([HD + 1, SUP], f32, tag=f"at{h}", name=f"at0_{h}")
        for h in range(HPC)
    ]
    ht_block(0)
    kv_block(0)
    q_proj(0)
    for b in range(1, NSUP):
        ht_block(b)
        for c in range(JS * (b - 1), JS * b):
            sc_exp(0, c)
            if c > 0:
                at_mms(0, c - 1)
        kv_block(b)
    for c in range(JS * (NSUP - 1), TC):
        sc_exp(0, c)
        at_mms(0, c - 1)
    q_proj(1)

    # ---- supers 1..7 with trailing C(sup-1), q_proj(sup+1) in-stream -----
    for sup in range(1, NSUP):
        slot = 0
        for c in range(TC):
            sc_exp(sup, c)
            if c == 0:
                at_mms(sup - 1, TC - 1)
                drain(sup - 1)
                at_of[sup] = [
                    at_ps_p.tile(
                        [HD + 1, SUP], f32, tag=f"at{h}", name=f"at{sup}_{h}"
                    )
                    for h in range(HPC)
                ]
            else:
                at_mms(sup, c - 1)
            if c % 3 == 2 and slot < 9:
                c_tail(sup - 1, slot)
                slot += 1
            if c == 29 and sup + 1 < NSUP:
                q_proj(sup + 1)
        while slot < 9:
            c_tail(sup - 1, slot)
            slot += 1
    at_mms(NSUP - 1, TC - 1)
    drain(NSUP - 1)
    for slot in range(9):
        c_tail(NSUP - 1, slot)

    st_.close()


def _shards(inputs):
    """Host-side prep: per-core input dicts (head-parallel, Wo row-shard)."""
    hs = np.ascontiguousarray(np.asarray(inputs["hidden_state"], np.float32))
    Wq = np.asarray(inputs["Wq"], np.float32) * 0.125  # fold 1/sqrt(64); exact
    bq = np.asarray(inputs["bq"], np.float32) * 0.125
    Wk = np.asarray(inputs["Wk"], np.float32)
    bk = np.asarray(inputs["bk"], np.float32)
    Wv = np.asarray(inputs["Wv"], np.float32)
    bv = np.asarray(inputs["bv"], np.float32)
    Wo = np.asarray(inputs["Wo"], np.float32)
    ident = np.eye(128, dtype=np.float32)
    hs_bf16 = None
    if PV_DT == "bf16":
        import ml_dtypes

        hs_bf16 = np.ascontiguousarray(hs.astype(ml_dtypes.bfloat16))

    in_maps = []
    for c in range(NCORES):
        h0 = HPC * c
        # [H,E,Dh] head-pair -> [E, 2*Dh] -> [EC, 128, 128]
        def _w(W):
            w = np.transpose(W[h0 : h0 + HPC], (1, 0, 2)).reshape(EMBED, 128)
            return np.ascontiguousarray(w.reshape(EC, 128, 128))

        w_qkv = np.stack([_w(Wq), _w(Wk), _w(Wv)])
        b_qk = np.stack(
            [bq[h0 : h0 + HPC].reshape(128), bk[h0 : h0 + HPC].reshape(128)]
        )
        b_v = np.ascontiguousarray(bv[h0 : h0 + HPC].reshape(128))
        # Wo rows for this core's heads: [128, E] -> [64, 2, E] (head-major)
        w_o = np.ascontiguousarray(
            Wo[128 * c : 128 * (c + 1)].reshape(2, 64, EMBED).transpose(1, 0, 2)
        )
        im = (
            {"hidden_bf16": hs_bf16} if PV_DT == "bf16" else {"hidden": hs}
        )
        in_maps.append(
            {
                **im,
                "w_qkv": np.ascontiguousarray(w_qkv),
                "b_qk": np.ascontiguousarray(b_qk),
                "b_v": b_v,
                "w_o": w_o,
                "ident": ident,
            }
        )
    return in_maps


def kernel(**inputs):
    global LAST
    from concourse import bass_utils

    trace = bool(int(os.environ.get("K_TRACE", "0")))
    if trace:
        _install_ntff_shim()

    key = (MM_DT, PV_DT)
    if key not in _CACHE:
        _CACHE[key] = _build()
    nc = _CACHE[key]

    in_maps = _shards(inputs)
    res = bass_utils.run_bass_kernel_spmd(
        nc, in_maps, core_ids=list(range(NCORES)), trace=trace
    )
    LAST = res

    out = np.zeros((SEQ, EMBED), np.float64)
    for c in range(NCORES):
        out += res.results[c]["out_p"].astype(np.float64)
    out += np.asarray(inputs["bo"], np.float32).astype(np.float64)
    return out.astype(np.float32)


def _install_ntff_shim():
    """antenv.axon_hooks is absent from this image; recreate it so
    run_bass_kernel_spmd(trace=True) can reach the NTFF profiling hook."""
    import types

    if "antenv.axon_hooks" in sys.modules:
        return
    try:
        if "/root/.axon_site" not in sys.path:
            sys.path.insert(0, "/root/.axon_site")
        from trn_agent_boot.trn_boot import _ntff_profile_via_ctypes

        hook = _ntff_profile_via_ctypes("/opt/axon/libaxon_pjrt.so")
    except Exception:
        hook = None
    mod = types.ModuleType("antenv.axon_hooks")
    mod._hook = hook
    mod.get_axon_ntff_profile_hook = lambda: mod._hook
    mod.set_axon_ntff_profile_hook = lambda h: setattr(mod, "_hook", h)
    sys.modules["antenv.axon_hooks"] = mod
